# revision 8
# baseline (speedup 1.0000x reference)
"""GCN encoder (2-layer) on 8 Trainium2 NeuronCores.

Math (per layer, matching the reference):
    out[d] = dis[d] * sum_{e: dst_e=d} dis[src_e] * h[src_e]  + b
with h = x @ W, dis = deg^-1/2 over src-with-self-loops. dis factors are
folded host-side: xT is pre-scaled by dis (layer-1 operand), layer-1's
output scaling uses dis^2 (post relu identity: dis*relu(z) = relu(dis*z)),
layer 2 applies dis at the end.

Design notes (this environment charges ~55us of fixed overhead per engine
instruction on top of data-proportional execution, so both instruction
count and gathered-token count are minimized):
  - dst nodes are assigned to [window, slot] positions sorted by in-degree
    (per core), so the per-window max rank K tracks the degree profile
    instead of the global max: padded gather tokens drop ~40%. Outputs are
    unpermuted host-side.
  - edges per dst-window (128 slots) are gathered in [rank, slot] order so
    token k*128+p is the k-th in-edge of window-slot p; one wide
    tensor_reduce over the rank axis aggregates a whole window pair.
  - per-window scale/bias ops are replaced by single whole-tensor
    scalar_tensor_tensor instructions using stride-0 broadcast APs.
  - dma_gather with single_packet=False allows ~8192 indices/instruction
    (single_packet=True hangs above ~1024).
  - rank padding points at injected all-zero rows: every core ships 6251
    rows (row 6250 zeroed), so zero rows exist in both the lo ([0,32768))
    and hi ([32768,50008)) gather bases of the int16-index split.
Sharding: nodes row-sharded 6250/core, edges partitioned by dst core,
weights replicated, AllGather between layers.
"""
import os
import numpy as np

N, E = 50000, 1600000
FIN, FHID, FOUT = 256, 128, 64
NCORES = 8
NPC = N // NCORES          # 6250
NPC2 = NPC + 1             # 6251 rows shipped per core (last = zeros)
NFULL = NCORES * NPC2      # 50008
NW = (NPC + 127) // 128    # 49 windows
NPAD = NW * 128            # 6272
HALF = 32768               # int16 gather base split
ZLO = 6250                 # zero row inside lo base (core 0 pad row)
ZHI = 5 * NPC2 + NPC - HALF  # core 5 pad row, hi-base-local index
MAXRANKS = 64              # ranks per gather instruction (8192 idxs)

_CACHE = {}
LAST_RESULTS = None


def _host_prep(x, edge_index, W1, b1, W2, b2):
    x = np.asarray(x, dtype=np.float32)
    ei = np.asarray(edge_index)
    W1 = np.asarray(W1, dtype=np.float32)
    W2 = np.asarray(W2, dtype=np.float32)
    b1 = np.asarray(b1, dtype=np.float32)
    b2 = np.asarray(b2, dtype=np.float32)

    loops = np.arange(N, dtype=np.int64)
    src0 = np.concatenate([ei[0].astype(np.int64), loops])
    dst0 = np.concatenate([ei[1].astype(np.int64), loops])

    deg = np.bincount(src0, minlength=N).astype(np.float32)
    dis0 = np.power(deg, np.float32(-0.5), dtype=np.float32)
    dis0[deg == 0] = 0.0

    # Relabel nodes within each core by in-degree (descending) so the
    # per-window max rank K tracks the degree profile instead of the
    # global max: padded gather tokens drop ~40%. Everything downstream
    # (xT, t1, t2, gidx, dis tables) lives in position space; outputs are
    # unpermuted host-side.
    deg_in = np.bincount(dst0, minlength=N)
    perms = []
    pos_of = np.empty(N, np.int64)
    for c in range(NCORES):
        oc = np.argsort(-deg_in[c * NPC:(c + 1) * NPC], kind="stable")
        perms.append(oc)
        pos_of[c * NPC:(c + 1) * NPC][oc] = np.arange(NPC)
    src = (src0 // NPC) * NPC + pos_of[src0]
    dst = (dst0 // NPC) * NPC + pos_of[dst0]
    dis = np.concatenate([dis0[c * NPC:(c + 1) * NPC][perms[c]]
                          for c in range(NCORES)])

    # padded gather row of each source node
    r_all = (src // NPC) * NPC2 + (src % NPC)
    s_all = (r_all >= HALF).astype(np.int64)  # 0 = lo stream, 1 = hi

    order = np.argsort(dst, kind="stable")
    r_s, dst_s, s_s = r_all[order], dst[order], s_all[order]
    cb = np.searchsorted(dst_s, np.arange(NCORES + 1) * NPC)

    # per-core rank assignment within (dst, stream)
    percore = []
    KLO = np.zeros((NCORES, NW), np.int64)
    KHI = np.zeros((NCORES, NW), np.int64)
    for c in range(NCORES):
        sl = slice(cb[c], cb[c + 1])
        r_c = r_s[sl]
        d_c = dst_s[sl] - c * NPC
        s_c = s_s[sl]
        key = d_c * 2 + s_c
        o2 = np.argsort(key, kind="stable")
        key_o = key[o2]
        first = np.searchsorted(key_o, key_o, side="left")
        rank = np.arange(len(key_o)) - first
        d_o, s_o, r_o = d_c[o2], s_c[o2], r_c[o2]
        w_o, p_o = d_o // 128, d_o % 128
        np.maximum.at(KLO[c], w_o[s_o == 0], rank[s_o == 0] + 1)
        np.maximum.at(KHI[c], w_o[s_o == 1], rank[s_o == 1] + 1)
        percore.append((w_o, p_o, s_o, rank, r_o))

    KLOm = KLO.max(axis=0)  # [NW]
    KHIm = KHI.max(axis=0)
    # pad window pairs (2w, 2w+1) to equal total ranks so one 4D-AP
    # tensor_reduce can aggregate both windows at once
    Kt = KLOm + KHIm
    for i in range(0, NW - 1, 2):
        kp = max(Kt[i], Kt[i + 1])
        KHIm[i] += kp - Kt[i]
        KHIm[i + 1] += kp - Kt[i + 1]
    K = KLOm + KHIm
    # flat token-position offsets: window w = [lo ranks][hi ranks]
    woff = np.zeros(NW + 1, np.int64)
    woff[1:] = np.cumsum(K) * 128
    total_tok = int(woff[-1])

    in_maps = []
    for c in range(NCORES):
        w_o, p_o, s_o, rank, r_o = percore[c]
        gidx = np.empty(total_tok, np.int16)
        for w in range(NW):
            gidx[woff[w]:woff[w] + KLOm[w] * 128] = ZLO
            gidx[woff[w] + KLOm[w] * 128:woff[w + 1]] = ZHI
        pos = woff[w_o] + (rank + np.where(s_o == 1, KLOm[w_o], 0)) * 128 + p_o
        gidx[pos] = np.where(s_o == 1, r_o - HALF, r_o).astype(np.int16)
        gidx_t = np.tile(gidx.reshape(-1, 16).T, (8, 1))  # [128, total_tok//16]

        dis_l = dis[c * NPC:(c + 1) * NPC]     # position space
        dis_pad = np.zeros(NPAD, np.float32)
        dis_pad[:NPC] = dis_l
        dis_col = np.ascontiguousarray(dis_pad.reshape(NW, 128).T)  # [128, NW]
        dis2_col = dis_col * dis_col
        # Bstt[p, w*128+f] = dis[w*128+p] * b1[f]
        Bstt = (dis_col.T[:, :, None] * b1[None, None, :]).transpose(1, 0, 2)
        Bstt = np.ascontiguousarray(Bstt.reshape(128, NW * FHID))

        xT = np.zeros((FIN, NPAD), np.float32)
        xT[:, :NPC] = (x[c * NPC + perms[c]] * dis_l[:, None]).T

        in_maps.append({
            "gidx": np.ascontiguousarray(gidx_t),
            "xT": xT,
            "W1": W1, "W2": W2,
            "dis2c": dis2_col, "disc": dis_col,
            "Bstt": Bstt,
            "b2b": np.tile(b2, (128, 1)),
            "ident": np.eye(128, dtype=np.float32),
        })
    return in_maps, (KLOm, KHIm, bool(not b1.any()), bool(not b2.any())), perms


def _build(Kinfo):
    import concourse.bacc as bacc
    import concourse.mybir as mybir
    import concourse.tile as tile

    KLOm, KHIm, B1ZERO, B2ZERO = Kinfo
    K = KLOm + KHIm
    maxK = max(int(K[i]) * (1 if i + 1 >= NW else 2)
               for i in range(0, NW, 2))
    total_tok = int(K.sum()) * 128

    PHASES = os.environ.get("GCN_PHASES", "full")
    REPEAT = int(os.environ.get("GCN_REPEAT", "1"))

    dt = mybir.dt
    ALU = mybir.AluOpType

    nc = bacc.Bacc("TRN2", target_bir_lowering=False, debug=False,
                   num_devices=NCORES)

    gidx_d = nc.dram_tensor("gidx", [128, total_tok // 16], dt.int16, kind="ExternalInput")
    xT_d = nc.dram_tensor("xT", [FIN, NPAD], dt.float32, kind="ExternalInput")
    W1_d = nc.dram_tensor("W1", [FIN, FHID], dt.float32, kind="ExternalInput")
    W2_d = nc.dram_tensor("W2", [FHID, FOUT], dt.float32, kind="ExternalInput")
    dis2_d = nc.dram_tensor("dis2c", [128, NW], dt.float32, kind="ExternalInput")
    dis_d = nc.dram_tensor("disc", [128, NW], dt.float32, kind="ExternalInput")
    Bstt_d = nc.dram_tensor("Bstt", [128, NW * FHID], dt.float32, kind="ExternalInput")
    b2b_d = nc.dram_tensor("b2b", [128, FOUT], dt.float32, kind="ExternalInput")
    ident_d = nc.dram_tensor("ident", [128, 128], dt.float32, kind="ExternalInput")
    out_d = nc.dram_tensor("out", [NPC, FOUT], dt.float32, kind="ExternalOutput")

    t1_local = nc.dram_tensor("t1_local", [NPC2, FHID], dt.float32)
    t1_full = nc.dram_tensor("t1_full", [NFULL, FHID], dt.float32, addr_space="Shared")
    t2_local = nc.dram_tensor("t2_local", [NPC2, FOUT], dt.float32)
    t2_full = nc.dram_tensor("t2_full", [NFULL, FOUT], dt.float32, addr_space="Shared")

    NWF = NW - 1  # 48 full windows; window 48 has 106 live rows

    with tile.TileContext(nc) as tc:
        with (
            tc.tile_pool(name="consts", bufs=1) as cp,
            tc.tile_pool(name="work", bufs=1) as wp,
            tc.tile_pool(name="psum", bufs=1, space="PSUM") as pp,
        ):
            ident_t = cp.tile([128, 128], dt.float32, tag="ident")
            nc.sync.dma_start(ident_t[:], ident_d[:, :])
            w1_t = cp.tile([128, 2, FHID], dt.float32, tag="w1")
            nc.sync.dma_start(w1_t[:, 0, :], W1_d[0:128, :])
            nc.sync.dma_start(w1_t[:, 1, :], W1_d[128:256, :])
            w2_t = cp.tile([FHID, FOUT], dt.float32, tag="w2")
            nc.sync.dma_start(w2_t[:], W2_d[:, :])
            dis2_t = cp.tile([128, NW], dt.float32, tag="dis2")
            nc.sync.dma_start(dis2_t[:], dis2_d[:, :])
            dis_t = cp.tile([128, NW], dt.float32, tag="dis")
            nc.sync.dma_start(dis_t[:], dis_d[:, :])
            if not B1ZERO:
                Bstt_t = cp.tile([128, NW * FHID], dt.float32, tag="Bstt")
                nc.sync.dma_start(Bstt_t[:], Bstt_d[:, :])
            b2b_t = cp.tile([128, FOUT], dt.float32, tag="b2b")
            nc.sync.dma_start(b2b_t[:], b2b_d[:, :])
            gidx_t = cp.tile([128, total_tok // 16], dt.int16, tag="gidx")
            nc.sync.dma_start(gidx_t[:], gidx_d[:, :])
            zrow = cp.tile([128, FHID], dt.float32, tag="zrow")
            nc.vector.memset(zrow[:], 0.0)

            # one shared gpsimd register per distinct gather count: avoids a
            # RegisterMove instruction (~55us here) per dma_gather
            counts = set()
            for w in range(NW):
                for nk in (int(KLOm[w]), int(KHIm[w])):
                    for k0 in range(0, nk, MAXRANKS):
                        counts.add(min(MAXRANKS, nk - k0) * 128)
            nidx_regs = {cnt: nc.gpsimd.to_reg(cnt) for cnt in sorted(counts)}

            dis2_bc = dis2_t[:].rearrange("p (w o) -> p w o", o=1) \
                               .broadcast_to([128, NW, FHID])
            dis_bc = dis_t[:].rearrange("p (w o) -> p w o", o=1) \
                             .broadcast_to([128, NW, FOUT])
            b2_bc = b2b_t[:].rearrange("(o p) f -> p o f", o=1) \
                            .broadcast_to([128, NW, FOUT])

            for _rep in range(REPEAT):
                # ---- phase B: t1_local = (dis*x) @ W1 ----
                with tc.tile_pool(name="phaseB", bufs=1) as pb:
                    xT_t = pb.tile([128, 2, NPAD], dt.float32, tag="xT")
                    nc.sync.dma_start(xT_t[:, 0, :], xT_d[0:128, :])
                    nc.sync.dma_start(xT_t[:, 1, :], xT_d[128:256, :])
                    evB = pb.tile([128, NWF, FHID], dt.float32, tag="evB")
                    evBt = pb.tile([128, FHID], dt.float32, tag="evBt")
                    psB = pp.tile([128, 8, FHID], dt.float32, tag="pB")
                    for w in range(NW):
                        sl = psB[:, w % 8, :]
                        for kc in range(2):
                            nc.tensor.matmul(
                                sl, xT_t[:, kc, w * 128:w * 128 + 128],
                                w1_t[:, kc, :], start=(kc == 0), stop=(kc == 1))
                        if w % 8 == 7:
                            nc.vector.tensor_copy(evB[:, w - 7:w + 1, :], psB[:])
                        if w == NW - 1:
                            nc.vector.tensor_copy(evBt[:], sl)
                    nc.sync.dma_start(
                        t1_local[0:NWF * 128, :]
                        .rearrange("(a p) f -> p a f", p=128), evB[:])
                    nc.sync.dma_start(t1_local[NWF * 128:NPC, :],
                                      evBt[0:NPC - NWF * 128, :])
                    nc.sync.dma_start(t1_local[NPC:NPC2, :], zrow[0:1, :])

                nc.gpsimd.collective_compute(
                    "AllGather", mybir.AluOpType.bypass,
                    replica_groups=[list(range(NCORES))],
                    ins=[t1_local[:, :]], outs=[t1_full[:, :]],
                )

                if PHASES == "Bdump":
                    # debug: out <- first FOUT cols of t1_local
                    nc.sync.dma_start(out_d[:, :], t1_local[0:NPC, 0:FOUT])
                    continue

                if PHASES == "B":
                    ot = wp.tile([128, FOUT], dt.float32, tag="o")
                    nc.vector.memset(ot[:], 0.0)
                    for w in range(NW):
                        rows = min(128, NPC - w * 128)
                        nc.sync.dma_start(out_d[w * 128:w * 128 + rows, :],
                                          ot[0:rows, :])
                    continue

                def gather_window(tok, w, src_full, feat, woff_w, dk=0):
                    """Emit gathers for window w into tok at rank offset dk."""
                    klo, khi = int(KLOm[w]), int(KHIm[w])
                    base_lo = src_full[0:HALF, :]
                    base_hi = src_full[HALF:NFULL, :]
                    segs = [(0, klo, base_lo), (klo, khi, base_hi)]
                    for seg0, nk, base in segs:
                        for k0 in range(0, nk, MAXRANKS):
                            kn = min(MAXRANKS, nk - k0)
                            c0 = (woff_w + (seg0 + k0) * 128) // 16
                            d0 = dk + seg0 + k0
                            nc.gpsimd.dma_gather(
                                tok[:, d0:d0 + kn, :], base,
                                gidx_t[:, c0:c0 + kn * 8],
                                num_idxs=kn * 128,
                                num_idxs_reg=nidx_regs[kn * 128],
                                elem_size=feat, single_packet=False)

                # ---- L1 pass 1: gather + reduce into red_all, one fused
                #      relu+scale over all windows ----
                with tc.tile_pool(name="L1", bufs=1) as l1:
                    tok = l1.tile([128, maxK, FHID], dt.float32, tag="tok1")
                    red_all = l1.tile([128, NW, FHID], dt.float32, tag="redA")
                    o1s_all = l1.tile([128, NW, FHID], dt.float32, tag="o1sa")
                    o1T = l1.tile([128, 4, FHID], dt.float32, tag="o1T")
                    ev1 = l1.tile([128, NWF, FOUT], dt.float32, tag="ev1")
                    ev1t = l1.tile([128, FOUT], dt.float32, tag="ev1t")
                    pT = pp.tile([128, 4, 512], dt.float32, tag="pT")  # slice per bank
                    p2 = pp.tile([128, 8, FOUT], dt.float32, tag="p2")
                    woff_w = 0
                    for w0 in range(0, NW, 2):
                        pair = [w0] if w0 + 1 >= NW else [w0, w0 + 1]
                        kp = int(K[w0])
                        for j, w in enumerate(pair):
                            gather_window(tok, w, t1_full, FHID,
                                          woff_w, j * kp)
                            woff_w += int(K[w]) * 128
                        nc.vector.tensor_reduce(
                            red_all[:, w0:w0 + len(pair), :],
                            tok[:, 0:len(pair) * kp, :]
                            .rearrange("p (b k) f -> p b f k", b=len(pair)),
                            mybir.AxisListType.X, ALU.add)
                    # o1s = relu(dis^2*red + dis*b1), batched over all windows
                    if B1ZERO:
                        if os.environ.get("GCN_NOBCAST"):
                            for w in range(NW):
                                nc.vector.tensor_scalar(
                                    o1s_all[:, w, :], red_all[:, w, :],
                                    dis2_t[:, w:w + 1], 0.0, ALU.mult, ALU.max)
                        else:
                            nc.vector.scalar_tensor_tensor(
                                o1s_all[:], red_all[:], 0.0, dis2_bc,
                                ALU.max, ALU.mult)
                    else:
                        nc.vector.scalar_tensor_tensor(
                            o1s_all[:], red_all[:], 0.0, dis2_bc,
                            ALU.bypass, ALU.mult)
                        nc.vector.scalar_tensor_tensor(
                            o1s_all[:], o1s_all[:], 0.0,
                            Bstt_t[:].rearrange("p (w f) -> p w f", w=NW),
                            ALU.bypass, ALU.add)
                        nc.vector.tensor_scalar(
                            o1s_all[:], o1s_all[:], 0.0, None, ALU.max)
                    # ---- L1 pass 2: transpose + @W2, batched ----
                    for w in range(NW):
                        nc.tensor.transpose(pT[:, w % 4, 0:FHID],
                                            o1s_all[:, w, :], ident_t[:])
                        if w % 4 == 3:
                            nc.vector.tensor_copy(o1T[:], pT[:, :, 0:FHID])
                        if w == NW - 1:
                            nc.vector.tensor_copy(o1T[:, 0, :], pT[:, 0, 0:FHID])
                        if w % 4 == 3 or w == NW - 1:
                            for w2 in range(w - (3 if w % 4 == 3 else 0), w + 1):
                                nc.tensor.matmul(p2[:, w2 % 8, :],
                                                 o1T[:, w2 % 4, :], w2_t[:],
                                                 start=True, stop=True)
                        if w % 8 == 7:
                            nc.vector.tensor_copy(ev1[:, w - 7:w + 1, :], p2[:])
                        if w == NW - 1:
                            nc.vector.tensor_copy(ev1t[:], p2[:, 0, :])
                    nc.sync.dma_start(
                        t2_local[0:NWF * 128, :]
                        .rearrange("(a p) f -> p a f", p=128), ev1[:])
                    nc.sync.dma_start(t2_local[NWF * 128:NPC, :],
                                      ev1t[0:NPC - NWF * 128, :])
                    nc.sync.dma_start(t2_local[NPC:NPC2, :], zrow[0:1, 0:FOUT])

                if PHASES == "B1dump":
                    nc.sync.dma_start(out_d[:, :], t2_local[0:NPC, :])
                    continue

                if PHASES == "B1":
                    ot = wp.tile([128, FOUT], dt.float32, tag="o")
                    nc.vector.memset(ot[:], 0.0)
                    for w in range(NW):
                        rows = min(128, NPC - w * 128)
                        nc.sync.dma_start(out_d[w * 128:w * 128 + rows, :],
                                          ot[0:rows, :])
                    continue

                nc.gpsimd.collective_compute(
                    "AllGather", mybir.AluOpType.bypass,
                    replica_groups=[list(range(NCORES))],
                    ins=[t2_local[:, :]], outs=[t2_full[:, :]],
                )

                # ---- L2 windows ----
                with tc.tile_pool(name="L2", bufs=1) as l2:
                    tok2 = l2.tile([128, maxK, FOUT], dt.float32, tag="tok2")
                    red2_all = l2.tile([128, NW, FOUT], dt.float32, tag="red2A")
                    ev2_all = l2.tile([128, NW, FOUT], dt.float32, tag="ev2A")
                    woff_w = 0
                    for w0 in range(0, NW, 2):
                        pair = [w0] if w0 + 1 >= NW else [w0, w0 + 1]
                        kp = int(K[w0])
                        for j, w in enumerate(pair):
                            gather_window(tok2, w, t2_full, FOUT,
                                          woff_w, j * kp)
                            woff_w += int(K[w]) * 128
                        nc.vector.tensor_reduce(
                            red2_all[:, w0:w0 + len(pair), :],
                            tok2[:, 0:len(pair) * kp, :]
                            .rearrange("p (b k) f -> p b f k", b=len(pair)),
                            mybir.AxisListType.X, ALU.add)
                    # out = dis*red2 (+ b2), batched over all windows
                    if os.environ.get("GCN_NOBCAST"):
                        for w in range(NW):
                            nc.vector.scalar_tensor_tensor(
                                ev2_all[:, w, :], red2_all[:, w, :],
                                dis_t[:, w:w + 1], b2b_t[:],
                                ALU.mult, ALU.add)
                    else:
                        nc.vector.scalar_tensor_tensor(
                            ev2_all[:], red2_all[:], 0.0, dis_bc,
                            ALU.bypass, ALU.mult)
                        if not B2ZERO:
                            nc.vector.scalar_tensor_tensor(
                                ev2_all[:], ev2_all[:], 0.0, b2_bc,
                                ALU.bypass, ALU.add)
                    nc.sync.dma_start(
                        out_d[0:NWF * 128, :]
                        .rearrange("(a p) f -> p a f", p=128),
                        ev2_all[:, 0:NWF, :])
                    nc.sync.dma_start(out_d[NWF * 128:NPC, :],
                                      ev2_all[0:NPC - NWF * 128, NWF, :])

    nc.compile()
    return nc


def kernel(x, edge_index, W1, b1, W2, b2):
    global LAST_RESULTS
    from concourse.bass_utils import run_bass_kernel_spmd

    in_maps, Kinfo, perms = _host_prep(x, edge_index, W1, b1, W2, b2)
    key = (Kinfo[0].tobytes(), Kinfo[1].tobytes(), Kinfo[2], Kinfo[3])
    if key not in _CACHE:
        _CACHE[key] = _build(Kinfo)
    nc = _CACHE[key]

    res = run_bass_kernel_spmd(nc, in_maps, list(range(NCORES)))
    LAST_RESULTS = res
    out = np.empty((N, FOUT), np.float32)
    for c in range(NCORES):
        out[c * NPC + perms[c]] = res.results[c]["out"]
    return out


# revision 18
# speedup vs baseline: 1.2933x; 1.2933x over previous
"""GCN encoder (2-layer) on 8 Trainium2 NeuronCores.

Math (per layer, matching the reference):
    out[d] = dis[d] * sum_{e: dst_e=d} dis[src_e] * h[src_e]  + b
with h = x @ W, dis = deg^-1/2 over src-with-self-loops. dis factors are
folded host-side: xT is pre-scaled by dis (layer-1 operand), layer-1's
output scaling uses dis^2 (post relu identity: dis*relu(z) = relu(dis*z)),
layer 2 applies dis at the end.

Design notes (this environment charges ~55us of fixed overhead per engine
instruction on top of data-proportional execution, so both instruction
count and gathered-token count are minimized):
  - dst nodes are assigned to [window, slot] positions sorted by in-degree
    (per core), so the per-window max rank K tracks the degree profile
    instead of the global max: padded gather tokens drop ~40%. Outputs are
    unpermuted host-side.
  - edges per dst-window (128 slots) are gathered in [rank, slot] order so
    token k*128+p is the k-th in-edge of window-slot p; one wide
    tensor_reduce over the rank axis aggregates a whole window pair.
  - per-window scale/bias ops are replaced by single whole-tensor
    scalar_tensor_tensor instructions using stride-0 broadcast APs.
  - gathers are issued as 1024-index single_packet instructions: the
    executor's per-token cost is ~12% lower than multi-packet mode and
    instruction count carries no measurable cost (single_packet=True
    hangs above ~1024 indices).
  - rank padding points at injected all-zero rows: every core ships 6251
    rows (row 6250 zeroed), so zero rows exist in both the lo ([0,32768))
    and hi ([17240,50008)) gather bases of the int16-index split. The two
    bases overlap: edges sourcing rows in the overlap are assigned to
    whichever stream balances the per-window rank counts.
Sharding: nodes row-sharded 6250/core, edges partitioned by dst core,
weights replicated, AllGather between layers.
"""
import os
import numpy as np

N, E = 50000, 1600000
FIN, FHID, FOUT = 256, 128, 64
NCORES = 8
NPC = N // NCORES          # 6250
NPC2 = NPC + 1             # 6251 rows shipped per core (last = zeros)
NFULL = NCORES * NPC2      # 50008
NW = (NPC + 127) // 128    # 49 windows
NPAD = NW * 128            # 6272
HALF = 32768               # lo gather base covers rows [0, HALF)
H2 = NFULL - HALF          # hi base covers rows [H2, NFULL); overlap rows
                           # [H2, HALF) may use either stream (flex)
ZLO = 6250                 # zero row inside lo base (core 0 pad row)
ZHI = 2 * NPC2 + NPC - H2  # core 2 pad row, hi-base-local index
MAXRANKS = 8               # ranks per gather instruction (1024 idxs);
                           # <=1024 idxs allows single_packet mode, which
                           # the executor runs ~12% faster per token

_CACHE = {}
LAST_RESULTS = None


def _host_prep(x, edge_index, W1, b1, W2, b2):
    x = np.asarray(x, dtype=np.float32)
    ei = np.asarray(edge_index)
    W1 = np.asarray(W1, dtype=np.float32)
    W2 = np.asarray(W2, dtype=np.float32)
    b1 = np.asarray(b1, dtype=np.float32)
    b2 = np.asarray(b2, dtype=np.float32)

    loops = np.arange(N, dtype=np.int64)
    src0 = np.concatenate([ei[0].astype(np.int64), loops])
    dst0 = np.concatenate([ei[1].astype(np.int64), loops])

    deg = np.bincount(src0, minlength=N).astype(np.float32)
    dis0 = np.power(deg, np.float32(-0.5), dtype=np.float32)
    dis0[deg == 0] = 0.0

    # Relabel nodes within each core by in-degree (descending) so the
    # per-window max rank K tracks the degree profile instead of the
    # global max: padded gather tokens drop ~40%. Everything downstream
    # (xT, t1, t2, gidx, dis tables) lives in position space; outputs are
    # unpermuted host-side.
    deg_in = np.bincount(dst0, minlength=N)
    perms = []
    pos_of = np.empty(N, np.int64)
    for c in range(NCORES):
        oc = np.argsort(-deg_in[c * NPC:(c + 1) * NPC], kind="stable")
        perms.append(oc)
        pos_of[c * NPC:(c + 1) * NPC][oc] = np.arange(NPC)
    src = (src0 // NPC) * NPC + pos_of[src0]
    dst = (dst0 // NPC) * NPC + pos_of[dst0]
    dis = np.concatenate([dis0[c * NPC:(c + 1) * NPC][perms[c]]
                          for c in range(NCORES)])

    # padded gather row of each source node
    r_all = (src // NPC) * NPC2 + (src % NPC)

    order = np.argsort(dst, kind="stable")
    r_s, dst_s = r_all[order], dst[order]
    cb = np.searchsorted(dst_s, np.arange(NCORES + 1) * NPC)

    # Stream classes: rows < H2 must use the lo base, rows >= HALF must
    # use the hi base, rows in [H2, HALF) may use either. Per (window,
    # slot), flexible edges are assigned to balance the two streams, so
    # the window rank count K tracks the max in-degree instead of
    # max(lo-degree) + max(hi-degree).
    wq_all = np.arange(NPC) // 128
    DegM = np.zeros(NW, np.int64)   # max slot degree per window
    LoM = np.zeros(NW, np.int64)    # max slot lo_only count
    HiM = np.zeros(NW, np.int64)    # max slot hi_only count
    pre = []
    for c in range(NCORES):
        sl = slice(cb[c], cb[c + 1])
        r_c = r_s[sl]
        d_c = dst_s[sl] - c * NPC
        # class: 0 = hi_only, 1 = flex, 2 = lo_only (order = hi fill pref)
        cls = np.where(r_c >= HALF, 0, np.where(r_c < H2, 2, 1))
        deg_q = np.bincount(d_c, minlength=NPC)
        hi_q = np.bincount(d_c[cls == 0], minlength=NPC)
        lo_q = np.bincount(d_c[cls == 2], minlength=NPC)
        np.maximum.at(DegM, wq_all, deg_q)
        np.maximum.at(LoM, wq_all, lo_q)
        np.maximum.at(HiM, wq_all, hi_q)
        pre.append((r_c, d_c, cls, deg_q, hi_q))
    Kw = np.maximum(DegM, LoM + HiM)
    KHIm = HiM.copy()               # hi segment = exactly HiM ranks
    KLOm = Kw - HiM                 # lo segment absorbs the slack

    percore = []
    for c in range(NCORES):
        r_c, d_c, cls, deg_q, hi_q = pre[c]
        # per-slot hi-stream target: all hi_only plus enough flex to keep
        # the lo count within KLOm
        h_q = np.maximum(hi_q, deg_q - KLOm[wq_all])
        key = d_c * 4 + cls
        o2 = np.argsort(key, kind="stable")
        d_o, r_o = d_c[o2], r_c[o2]
        grp = np.searchsorted(d_o, d_o, side="left")
        i_in = np.arange(len(d_o)) - grp     # index within slot
        hq_o = h_q[d_o]
        s_o = (i_in < hq_o).astype(np.int64)  # 1 = hi stream
        rank = np.where(s_o == 1, i_in, i_in - hq_o)
        w_o, p_o = d_o // 128, d_o % 128
        percore.append((w_o, p_o, s_o, rank, r_o))
    # pad window pairs (2w, 2w+1) to equal total ranks so one 4D-AP
    # tensor_reduce can aggregate both windows at once
    Kt = KLOm + KHIm
    for i in range(0, NW - 1, 2):
        kp = max(Kt[i], Kt[i + 1])
        KHIm[i] += kp - Kt[i]
        KHIm[i + 1] += kp - Kt[i + 1]
    K = KLOm + KHIm
    # flat token-position offsets: window w = [lo ranks][hi ranks]
    woff = np.zeros(NW + 1, np.int64)
    woff[1:] = np.cumsum(K) * 128
    total_tok = int(woff[-1])

    in_maps = []
    for c in range(NCORES):
        w_o, p_o, s_o, rank, r_o = percore[c]
        gidx = np.empty(total_tok, np.int16)
        for w in range(NW):
            gidx[woff[w]:woff[w] + KLOm[w] * 128] = ZLO
            gidx[woff[w] + KLOm[w] * 128:woff[w + 1]] = ZHI
        pos = woff[w_o] + (rank + np.where(s_o == 1, KLOm[w_o], 0)) * 128 + p_o
        gidx[pos] = np.where(s_o == 1, r_o - H2, r_o).astype(np.int16)
        gidx_t = np.tile(gidx.reshape(-1, 16).T, (8, 1))  # [128, total_tok//16]

        dis_l = dis[c * NPC:(c + 1) * NPC]     # position space
        dis_pad = np.zeros(NPAD, np.float32)
        dis_pad[:NPC] = dis_l
        dis_col = np.ascontiguousarray(dis_pad.reshape(NW, 128).T)  # [128, NW]
        dis2_col = dis_col * dis_col
        # Bstt[p, w*128+f] = dis[w*128+p] * b1[f]
        Bstt = (dis_col.T[:, :, None] * b1[None, None, :]).transpose(1, 0, 2)
        Bstt = np.ascontiguousarray(Bstt.reshape(128, NW * FHID))

        xT = np.zeros((FIN, NPAD), np.float32)
        xT[:, :NPC] = (x[c * NPC + perms[c]] * dis_l[:, None]).T

        in_maps.append({
            "gidx": np.ascontiguousarray(gidx_t),
            "xT": xT,
            "W1": W1, "W2": W2,
            "dis2c": dis2_col, "disc": dis_col,
            "Bstt": Bstt,
            "b2b": np.tile(b2, (128, 1)),
            "ident": np.eye(128, dtype=np.float32),
        })
    return in_maps, (KLOm, KHIm, bool(not b1.any()), bool(not b2.any())), perms


def _build(Kinfo):
    import concourse.bacc as bacc
    import concourse.mybir as mybir
    import concourse.tile as tile

    KLOm, KHIm, B1ZERO, B2ZERO = Kinfo
    K = KLOm + KHIm
    maxK = max(int(K[i]) * (1 if i + 1 >= NW else 2)
               for i in range(0, NW, 2))
    total_tok = int(K.sum()) * 128

    PHASES = os.environ.get("GCN_PHASES", "full")
    REPEAT = int(os.environ.get("GCN_REPEAT", "1"))
    SKIPGATHER = bool(os.environ.get("GCN_SKIPGATHER"))  # timing probe only
    SKIPREDUCE = bool(os.environ.get("GCN_SKIPREDUCE"))  # timing probe only
    SKIPAG = bool(os.environ.get("GCN_SKIPAG"))          # timing probe only
    MR = int(os.environ.get("GCN_MAXRANKS", str(MAXRANKS)))

    dt = mybir.dt
    ALU = mybir.AluOpType

    nc = bacc.Bacc("TRN2", target_bir_lowering=False, debug=False,
                   num_devices=NCORES)

    gidx_d = nc.dram_tensor("gidx", [128, total_tok // 16], dt.int16, kind="ExternalInput")
    xT_d = nc.dram_tensor("xT", [FIN, NPAD], dt.float32, kind="ExternalInput")
    W1_d = nc.dram_tensor("W1", [FIN, FHID], dt.float32, kind="ExternalInput")
    W2_d = nc.dram_tensor("W2", [FHID, FOUT], dt.float32, kind="ExternalInput")
    dis2_d = nc.dram_tensor("dis2c", [128, NW], dt.float32, kind="ExternalInput")
    dis_d = nc.dram_tensor("disc", [128, NW], dt.float32, kind="ExternalInput")
    Bstt_d = nc.dram_tensor("Bstt", [128, NW * FHID], dt.float32, kind="ExternalInput")
    b2b_d = nc.dram_tensor("b2b", [128, FOUT], dt.float32, kind="ExternalInput")
    ident_d = nc.dram_tensor("ident", [128, 128], dt.float32, kind="ExternalInput")
    out_d = nc.dram_tensor("out", [NPC, FOUT], dt.float32, kind="ExternalOutput")

    t1_local = nc.dram_tensor("t1_local", [NPC2, FHID], dt.float32)
    t1_full = nc.dram_tensor("t1_full", [NFULL, FHID], dt.float32, addr_space="Shared")
    t2_local = nc.dram_tensor("t2_local", [NPC2, FOUT], dt.float32)
    t2_full = nc.dram_tensor("t2_full", [NFULL, FOUT], dt.float32, addr_space="Shared")

    NWF = NW - 1  # 48 full windows; window 48 has 106 live rows

    with tile.TileContext(nc) as tc:
        with (
            tc.tile_pool(name="consts", bufs=1) as cp,
            tc.tile_pool(name="work", bufs=1) as wp,
            tc.tile_pool(name="psum", bufs=1, space="PSUM") as pp,
        ):
            ident_t = cp.tile([128, 128], dt.float32, tag="ident")
            nc.sync.dma_start(ident_t[:], ident_d[:, :])
            w1_t = cp.tile([128, 2, FHID], dt.float32, tag="w1")
            nc.sync.dma_start(w1_t[:, 0, :], W1_d[0:128, :])
            nc.sync.dma_start(w1_t[:, 1, :], W1_d[128:256, :])
            w2_t = cp.tile([FHID, FOUT], dt.float32, tag="w2")
            nc.sync.dma_start(w2_t[:], W2_d[:, :])
            dis2_t = cp.tile([128, NW], dt.float32, tag="dis2")
            nc.sync.dma_start(dis2_t[:], dis2_d[:, :])
            dis_t = cp.tile([128, NW], dt.float32, tag="dis")
            nc.sync.dma_start(dis_t[:], dis_d[:, :])
            if not B1ZERO:
                Bstt_t = cp.tile([128, NW * FHID], dt.float32, tag="Bstt")
                nc.sync.dma_start(Bstt_t[:], Bstt_d[:, :])
            b2b_t = cp.tile([128, FOUT], dt.float32, tag="b2b")
            nc.sync.dma_start(b2b_t[:], b2b_d[:, :])
            gidx_t = cp.tile([128, total_tok // 16], dt.int16, tag="gidx")
            nc.sync.dma_start(gidx_t[:], gidx_d[:, :])
            zrow = cp.tile([128, FHID], dt.float32, tag="zrow")
            nc.vector.memset(zrow[:], 0.0)

            # one shared gpsimd register per distinct gather count: avoids a
            # RegisterMove instruction (~55us here) per dma_gather
            counts = set()
            for w in range(NW):
                for nk in (int(KLOm[w]), int(KHIm[w])):
                    for k0 in range(0, nk, MR):
                        counts.add(min(MR, nk - k0) * 128)
            nidx_regs = {cnt: nc.gpsimd.to_reg(cnt) for cnt in sorted(counts)}

            dis2_bc = dis2_t[:].rearrange("p (w o) -> p w o", o=1) \
                               .broadcast_to([128, NW, FHID])
            dis_bc = dis_t[:].rearrange("p (w o) -> p w o", o=1) \
                             .broadcast_to([128, NW, FOUT])
            b2_bc = b2b_t[:].rearrange("(o p) f -> p o f", o=1) \
                            .broadcast_to([128, NW, FOUT])

            for _rep in range(REPEAT):
                # ---- phase B: t1_local = (dis*x) @ W1 ----
                with tc.tile_pool(name="phaseB", bufs=1) as pb:
                    xT_t = pb.tile([128, 2, NPAD], dt.float32, tag="xT")
                    nc.sync.dma_start(xT_t[:, 0, :], xT_d[0:128, :])
                    nc.sync.dma_start(xT_t[:, 1, :], xT_d[128:256, :])
                    evB = pb.tile([128, NWF, FHID], dt.float32, tag="evB")
                    evBt = pb.tile([128, FHID], dt.float32, tag="evBt")
                    psB = pp.tile([128, 8, FHID], dt.float32, tag="pB")
                    for w in range(NW):
                        sl = psB[:, w % 8, :]
                        for kc in range(2):
                            nc.tensor.matmul(
                                sl, xT_t[:, kc, w * 128:w * 128 + 128],
                                w1_t[:, kc, :], start=(kc == 0), stop=(kc == 1))
                        if w % 8 == 7:
                            nc.vector.tensor_copy(evB[:, w - 7:w + 1, :], psB[:])
                        if w == NW - 1:
                            nc.vector.tensor_copy(evBt[:], sl)
                    nc.sync.dma_start(
                        t1_local[0:NWF * 128, :]
                        .rearrange("(a p) f -> p a f", p=128), evB[:])
                    nc.sync.dma_start(t1_local[NWF * 128:NPC, :],
                                      evBt[0:NPC - NWF * 128, :])
                    nc.sync.dma_start(t1_local[NPC:NPC2, :], zrow[0:1, :])

                if SKIPAG:
                    nc.sync.dma_start(t1_full[0:NPC2, :], t1_local[:, :])
                else:
                    nc.gpsimd.collective_compute(
                        "AllGather", mybir.AluOpType.bypass,
                        replica_groups=[list(range(NCORES))],
                        ins=[t1_local[:, :]], outs=[t1_full[:, :]],
                    )

                if PHASES == "Bdump":
                    # debug: out <- first FOUT cols of t1_local
                    nc.sync.dma_start(out_d[:, :], t1_local[0:NPC, 0:FOUT])
                    continue

                if PHASES == "B":
                    ot = wp.tile([128, FOUT], dt.float32, tag="o")
                    nc.vector.memset(ot[:], 0.0)
                    for w in range(NW):
                        rows = min(128, NPC - w * 128)
                        nc.sync.dma_start(out_d[w * 128:w * 128 + rows, :],
                                          ot[0:rows, :])
                    continue

                def gather_window(tok, w, src_full, feat, woff_w, dk=0):
                    """Emit gathers for window w into tok at rank offset dk."""
                    klo, khi = int(KLOm[w]), int(KHIm[w])
                    base_lo = src_full[0:HALF, :]
                    base_hi = src_full[H2:NFULL, :]
                    segs = [(0, klo, base_lo), (klo, khi, base_hi)]
                    for seg0, nk, base in segs:
                        for k0 in range(0, nk, MR):
                            kn = min(MR, nk - k0)
                            c0 = (woff_w + (seg0 + k0) * 128) // 16
                            d0 = dk + seg0 + k0
                            if SKIPGATHER:
                                continue
                            nc.gpsimd.dma_gather(
                                tok[:, d0:d0 + kn, :], base,
                                gidx_t[:, c0:c0 + kn * 8],
                                num_idxs=kn * 128,
                                num_idxs_reg=nidx_regs[kn * 128],
                                elem_size=feat,
                                single_packet=(kn * 128 <= 1024))

                # ---- L1 pass 1: gather + reduce into red_all, one fused
                #      relu+scale over all windows ----
                with tc.tile_pool(name="L1", bufs=1) as l1:
                    tok = l1.tile([128, maxK, FHID], dt.float32, tag="tok1")
                    red_all = l1.tile([128, NW, FHID], dt.float32, tag="redA")
                    o1s_all = l1.tile([128, NW, FHID], dt.float32, tag="o1sa")
                    o1T = l1.tile([128, 4, FHID], dt.float32, tag="o1T")
                    ev1 = l1.tile([128, NWF, FOUT], dt.float32, tag="ev1")
                    ev1t = l1.tile([128, FOUT], dt.float32, tag="ev1t")
                    pT = pp.tile([128, 4, 512], dt.float32, tag="pT")  # slice per bank
                    p2 = pp.tile([128, 8, FOUT], dt.float32, tag="p2")
                    if SKIPGATHER:
                        nc.vector.memset(tok[:], 0.0)
                    if SKIPREDUCE:
                        nc.vector.memset(red_all[:], 0.0)
                    woff_w = 0
                    for w0 in range(0, NW, 2):
                        pair = [w0] if w0 + 1 >= NW else [w0, w0 + 1]
                        kp = int(K[w0])
                        for j, w in enumerate(pair):
                            gather_window(tok, w, t1_full, FHID,
                                          woff_w, j * kp)
                            woff_w += int(K[w]) * 128
                        if not SKIPREDUCE:
                            nc.vector.tensor_reduce(
                                red_all[:, w0:w0 + len(pair), :],
                                tok[:, 0:len(pair) * kp, :]
                                .rearrange("p (b k) f -> p b f k", b=len(pair)),
                                mybir.AxisListType.X, ALU.add)
                    # o1s = relu(dis^2*red + dis*b1), batched over all windows
                    if B1ZERO:
                        if os.environ.get("GCN_NOBCAST"):
                            for w in range(NW):
                                nc.vector.tensor_scalar(
                                    o1s_all[:, w, :], red_all[:, w, :],
                                    dis2_t[:, w:w + 1], 0.0, ALU.mult, ALU.max)
                        else:
                            nc.vector.scalar_tensor_tensor(
                                o1s_all[:], red_all[:], 0.0, dis2_bc,
                                ALU.max, ALU.mult)
                    else:
                        nc.vector.scalar_tensor_tensor(
                            o1s_all[:], red_all[:], 0.0, dis2_bc,
                            ALU.bypass, ALU.mult)
                        nc.vector.scalar_tensor_tensor(
                            o1s_all[:], o1s_all[:], 0.0,
                            Bstt_t[:].rearrange("p (w f) -> p w f", w=NW),
                            ALU.bypass, ALU.add)
                        nc.vector.tensor_scalar(
                            o1s_all[:], o1s_all[:], 0.0, None, ALU.max)
                    # ---- L1 pass 2: transpose + @W2, batched ----
                    for w in range(NW):
                        nc.tensor.transpose(pT[:, w % 4, 0:FHID],
                                            o1s_all[:, w, :], ident_t[:])
                        if w % 4 == 3:
                            nc.vector.tensor_copy(o1T[:], pT[:, :, 0:FHID])
                        if w == NW - 1:
                            nc.vector.tensor_copy(o1T[:, 0, :], pT[:, 0, 0:FHID])
                        if w % 4 == 3 or w == NW - 1:
                            for w2 in range(w - (3 if w % 4 == 3 else 0), w + 1):
                                nc.tensor.matmul(p2[:, w2 % 8, :],
                                                 o1T[:, w2 % 4, :], w2_t[:],
                                                 start=True, stop=True)
                        if w % 8 == 7:
                            nc.vector.tensor_copy(ev1[:, w - 7:w + 1, :], p2[:])
                        if w == NW - 1:
                            nc.vector.tensor_copy(ev1t[:], p2[:, 0, :])
                    nc.sync.dma_start(
                        t2_local[0:NWF * 128, :]
                        .rearrange("(a p) f -> p a f", p=128), ev1[:])
                    nc.sync.dma_start(t2_local[NWF * 128:NPC, :],
                                      ev1t[0:NPC - NWF * 128, :])
                    nc.sync.dma_start(t2_local[NPC:NPC2, :], zrow[0:1, 0:FOUT])

                if PHASES == "B1dump":
                    nc.sync.dma_start(out_d[:, :], t2_local[0:NPC, :])
                    continue

                if PHASES == "B1":
                    ot = wp.tile([128, FOUT], dt.float32, tag="o")
                    nc.vector.memset(ot[:], 0.0)
                    for w in range(NW):
                        rows = min(128, NPC - w * 128)
                        nc.sync.dma_start(out_d[w * 128:w * 128 + rows, :],
                                          ot[0:rows, :])
                    continue

                if SKIPAG:
                    nc.sync.dma_start(t2_full[0:NPC2, :], t2_local[:, :])
                else:
                    nc.gpsimd.collective_compute(
                        "AllGather", mybir.AluOpType.bypass,
                        replica_groups=[list(range(NCORES))],
                        ins=[t2_local[:, :]], outs=[t2_full[:, :]],
                    )

                # ---- L2 windows ----
                with tc.tile_pool(name="L2", bufs=1) as l2:
                    tok2 = l2.tile([128, maxK, FOUT], dt.float32, tag="tok2")
                    red2_all = l2.tile([128, NW, FOUT], dt.float32, tag="red2A")
                    ev2_all = l2.tile([128, NW, FOUT], dt.float32, tag="ev2A")
                    if SKIPGATHER:
                        nc.vector.memset(tok2[:], 0.0)
                    if SKIPREDUCE:
                        nc.vector.memset(red2_all[:], 0.0)
                    woff_w = 0
                    for w0 in range(0, NW, 2):
                        pair = [w0] if w0 + 1 >= NW else [w0, w0 + 1]
                        kp = int(K[w0])
                        for j, w in enumerate(pair):
                            gather_window(tok2, w, t2_full, FOUT,
                                          woff_w, j * kp)
                            woff_w += int(K[w]) * 128
                        if not SKIPREDUCE:
                            nc.vector.tensor_reduce(
                                red2_all[:, w0:w0 + len(pair), :],
                                tok2[:, 0:len(pair) * kp, :]
                                .rearrange("p (b k) f -> p b f k", b=len(pair)),
                                mybir.AxisListType.X, ALU.add)
                    # out = dis*red2 (+ b2), batched over all windows
                    if os.environ.get("GCN_NOBCAST"):
                        for w in range(NW):
                            nc.vector.scalar_tensor_tensor(
                                ev2_all[:, w, :], red2_all[:, w, :],
                                dis_t[:, w:w + 1], b2b_t[:],
                                ALU.mult, ALU.add)
                    else:
                        nc.vector.scalar_tensor_tensor(
                            ev2_all[:], red2_all[:], 0.0, dis_bc,
                            ALU.bypass, ALU.mult)
                        if not B2ZERO:
                            nc.vector.scalar_tensor_tensor(
                                ev2_all[:], ev2_all[:], 0.0, b2_bc,
                                ALU.bypass, ALU.add)
                    nc.sync.dma_start(
                        out_d[0:NWF * 128, :]
                        .rearrange("(a p) f -> p a f", p=128),
                        ev2_all[:, 0:NWF, :])
                    nc.sync.dma_start(out_d[NWF * 128:NPC, :],
                                      ev2_all[0:NPC - NWF * 128, NWF, :])

    nc.compile()
    return nc


def kernel(x, edge_index, W1, b1, W2, b2):
    global LAST_RESULTS
    from concourse.bass_utils import run_bass_kernel_spmd

    in_maps, Kinfo, perms = _host_prep(x, edge_index, W1, b1, W2, b2)
    key = (Kinfo[0].tobytes(), Kinfo[1].tobytes(), Kinfo[2], Kinfo[3])
    if key not in _CACHE:
        _CACHE[key] = _build(Kinfo)
    nc = _CACHE[key]

    res = run_bass_kernel_spmd(nc, in_maps, list(range(NCORES)))
    LAST_RESULTS = res
    out = np.empty((N, FOUT), np.float32)
    for c in range(NCORES):
        out[c * NPC + perms[c]] = res.results[c]["out"]
    return out


# revision 20
# speedup vs baseline: 1.3268x; 1.0259x over previous
"""GCN encoder (2-layer) on 8 Trainium2 NeuronCores.

Math (per layer, matching the reference):
    out[d] = dis[d] * sum_{e: dst_e=d} dis[src_e] * h[src_e]  + b
with h = x @ W, dis = deg^-1/2 over src-with-self-loops. dis factors are
folded host-side: xT is pre-scaled by dis (layer-1 operand), layer-1's
output scaling uses dis^2 (post relu identity: dis*relu(z) = relu(dis*z)),
layer 2 applies dis at the end.

Design notes (this environment charges ~55us of fixed overhead per engine
instruction on top of data-proportional execution, so both instruction
count and gathered-token count are minimized):
  - dst nodes are assigned to [window, slot] positions sorted by in-degree
    (per core), so the per-window max rank K tracks the degree profile
    instead of the global max: padded gather tokens drop ~40%. Outputs are
    unpermuted host-side.
  - edges per dst-window (128 slots) are gathered in [rank, slot] order so
    token k*128+p is the k-th in-edge of window-slot p; one wide
    tensor_reduce over the rank axis aggregates a whole window pair.
  - per-window scale/bias ops are replaced by single whole-tensor
    scalar_tensor_tensor instructions using stride-0 broadcast APs.
  - gathers are issued as 1024-index single_packet instructions: the
    executor's per-token cost is ~12% lower than multi-packet mode and
    instruction count carries no measurable cost (single_packet=True
    hangs above ~1024 indices).
  - rank padding points at injected all-zero rows: every core ships 6251
    rows (row 6250 zeroed), so zero rows exist in both the lo ([0,32768))
    and hi ([17240,50008)) gather bases of the int16-index split. The two
    bases overlap: edges sourcing rows in the overlap are assigned to
    whichever stream balances the per-window rank counts.
Sharding: nodes row-sharded 6250/core, edges partitioned by dst core,
weights replicated, AllGather between layers.
"""
import os
import numpy as np

N, E = 50000, 1600000
FIN, FHID, FOUT = 256, 128, 64
NCORES = 8
NPC = N // NCORES          # 6250
NPC2 = NPC + 1             # 6251 rows shipped per core (last = zeros)
NFULL = NCORES * NPC2      # 50008
NW = (NPC + 127) // 128    # 49 windows
NPAD = NW * 128            # 6272
HALF = 32768               # lo gather base covers rows [0, HALF)
H2 = NFULL - HALF          # hi base covers rows [H2, NFULL); overlap rows
                           # [H2, HALF) may use either stream (flex)
ZLO = 6250                 # zero row inside lo base (core 0 pad row)
ZHI = 2 * NPC2 + NPC - H2  # core 2 pad row, hi-base-local index
MAXRANKS = 8               # ranks per gather instruction (1024 idxs);
                           # <=1024 idxs allows single_packet mode, which
                           # the executor runs ~12% faster per token

_CACHE = {}
LAST_RESULTS = None


def _host_prep(x, edge_index, W1, b1, W2, b2):
    x = np.asarray(x, dtype=np.float32)
    ei = np.asarray(edge_index)
    W1 = np.asarray(W1, dtype=np.float32)
    W2 = np.asarray(W2, dtype=np.float32)
    b1 = np.asarray(b1, dtype=np.float32)
    b2 = np.asarray(b2, dtype=np.float32)

    loops = np.arange(N, dtype=np.int64)
    src0 = np.concatenate([ei[0].astype(np.int64), loops])
    dst0 = np.concatenate([ei[1].astype(np.int64), loops])

    deg = np.bincount(src0, minlength=N).astype(np.float32)
    dis0 = np.power(deg, np.float32(-0.5), dtype=np.float32)
    dis0[deg == 0] = 0.0

    # Relabel nodes within each core by in-degree (descending) so the
    # per-window max rank K tracks the degree profile instead of the
    # global max: padded gather tokens drop ~40%. Everything downstream
    # (xT, t1, t2, gidx, dis tables) lives in position space; outputs are
    # unpermuted host-side.
    deg_in = np.bincount(dst0, minlength=N)
    perms = []
    pos_of = np.empty(N, np.int64)
    for c in range(NCORES):
        oc = np.argsort(-deg_in[c * NPC:(c + 1) * NPC], kind="stable")
        perms.append(oc)
        pos_of[c * NPC:(c + 1) * NPC][oc] = np.arange(NPC)
    # Self-loops are excluded from the gather pipeline: the kernel adds each
    # node's own t1/t2 row directly (an aligned DMA load + one add), which
    # removes 6250 real tokens per core per layer and lowers every window's
    # max rank by one. dis still uses the with-loops degree (reference).
    srcE = ei[0].astype(np.int64)
    dstE = ei[1].astype(np.int64)
    src = (srcE // NPC) * NPC + pos_of[srcE]
    dst = (dstE // NPC) * NPC + pos_of[dstE]
    dis = np.concatenate([dis0[c * NPC:(c + 1) * NPC][perms[c]]
                          for c in range(NCORES)])

    # padded gather row of each source node
    r_all = (src // NPC) * NPC2 + (src % NPC)

    order = np.argsort(dst, kind="stable")
    r_s, dst_s = r_all[order], dst[order]
    cb = np.searchsorted(dst_s, np.arange(NCORES + 1) * NPC)

    # Stream classes: rows < H2 must use the lo base, rows >= HALF must
    # use the hi base, rows in [H2, HALF) may use either. Per (window,
    # slot), flexible edges are assigned to balance the two streams, so
    # the window rank count K tracks the max in-degree instead of
    # max(lo-degree) + max(hi-degree).
    wq_all = np.arange(NPC) // 128
    DegM = np.zeros(NW, np.int64)   # max slot degree per window
    LoM = np.zeros(NW, np.int64)    # max slot lo_only count
    HiM = np.zeros(NW, np.int64)    # max slot hi_only count
    pre = []
    for c in range(NCORES):
        sl = slice(cb[c], cb[c + 1])
        r_c = r_s[sl]
        d_c = dst_s[sl] - c * NPC
        # class: 0 = hi_only, 1 = flex, 2 = lo_only (order = hi fill pref)
        cls = np.where(r_c >= HALF, 0, np.where(r_c < H2, 2, 1))
        deg_q = np.bincount(d_c, minlength=NPC)
        hi_q = np.bincount(d_c[cls == 0], minlength=NPC)
        lo_q = np.bincount(d_c[cls == 2], minlength=NPC)
        np.maximum.at(DegM, wq_all, deg_q)
        np.maximum.at(LoM, wq_all, lo_q)
        np.maximum.at(HiM, wq_all, hi_q)
        pre.append((r_c, d_c, cls, deg_q, hi_q))
    Kw = np.maximum(DegM, LoM + HiM)
    KHIm = HiM.copy()               # hi segment = exactly HiM ranks
    KLOm = Kw - HiM                 # lo segment absorbs the slack

    percore = []
    for c in range(NCORES):
        r_c, d_c, cls, deg_q, hi_q = pre[c]
        # per-slot hi-stream target: all hi_only plus enough flex to keep
        # the lo count within KLOm
        h_q = np.maximum(hi_q, deg_q - KLOm[wq_all])
        key = d_c * 4 + cls
        o2 = np.argsort(key, kind="stable")
        d_o, r_o = d_c[o2], r_c[o2]
        grp = np.searchsorted(d_o, d_o, side="left")
        i_in = np.arange(len(d_o)) - grp     # index within slot
        hq_o = h_q[d_o]
        s_o = (i_in < hq_o).astype(np.int64)  # 1 = hi stream
        rank = np.where(s_o == 1, i_in, i_in - hq_o)
        w_o, p_o = d_o // 128, d_o % 128
        percore.append((w_o, p_o, s_o, rank, r_o))
    # pad window pairs (2w, 2w+1) to equal total ranks so one 4D-AP
    # tensor_reduce can aggregate both windows at once
    Kt = KLOm + KHIm
    for i in range(0, NW - 1, 2):
        kp = max(Kt[i], Kt[i + 1])
        KHIm[i] += kp - Kt[i]
        KHIm[i + 1] += kp - Kt[i + 1]
    K = KLOm + KHIm
    # flat token-position offsets: window w = [lo ranks][hi ranks]
    woff = np.zeros(NW + 1, np.int64)
    woff[1:] = np.cumsum(K) * 128
    total_tok = int(woff[-1])

    in_maps = []
    for c in range(NCORES):
        w_o, p_o, s_o, rank, r_o = percore[c]
        gidx = np.empty(total_tok, np.int16)
        for w in range(NW):
            gidx[woff[w]:woff[w] + KLOm[w] * 128] = ZLO
            gidx[woff[w] + KLOm[w] * 128:woff[w + 1]] = ZHI
        pos = woff[w_o] + (rank + np.where(s_o == 1, KLOm[w_o], 0)) * 128 + p_o
        gidx[pos] = np.where(s_o == 1, r_o - H2, r_o).astype(np.int16)
        gidx_t = np.tile(gidx.reshape(-1, 16).T, (8, 1))  # [128, total_tok//16]

        dis_l = dis[c * NPC:(c + 1) * NPC]     # position space
        dis_pad = np.zeros(NPAD, np.float32)
        dis_pad[:NPC] = dis_l
        dis_col = np.ascontiguousarray(dis_pad.reshape(NW, 128).T)  # [128, NW]
        dis2_col = dis_col * dis_col
        # Bstt[p, w*128+f] = dis[w*128+p] * b1[f]
        Bstt = (dis_col.T[:, :, None] * b1[None, None, :]).transpose(1, 0, 2)
        Bstt = np.ascontiguousarray(Bstt.reshape(128, NW * FHID))

        xT = np.zeros((FIN, NPAD), np.float32)
        xT[:, :NPC] = (x[c * NPC + perms[c]] * dis_l[:, None]).T

        in_maps.append({
            "gidx": np.ascontiguousarray(gidx_t),
            "xT": xT,
            "W1": W1, "W2": W2,
            "dis2c": dis2_col, "disc": dis_col,
            "Bstt": Bstt,
            "b2b": np.tile(b2, (128, 1)),
            "ident": np.eye(128, dtype=np.float32),
        })
    return in_maps, (KLOm, KHIm, bool(not b1.any()), bool(not b2.any())), perms


def _build(Kinfo):
    import concourse.bacc as bacc
    import concourse.mybir as mybir
    import concourse.tile as tile

    KLOm, KHIm, B1ZERO, B2ZERO = Kinfo
    K = KLOm + KHIm
    maxK = max(int(K[i]) * (1 if i + 1 >= NW else 2)
               for i in range(0, NW, 2))
    total_tok = int(K.sum()) * 128

    PHASES = os.environ.get("GCN_PHASES", "full")
    REPEAT = int(os.environ.get("GCN_REPEAT", "1"))
    SKIPGATHER = bool(os.environ.get("GCN_SKIPGATHER"))  # timing probe only
    SKIPREDUCE = bool(os.environ.get("GCN_SKIPREDUCE"))  # timing probe only
    SKIPAG = bool(os.environ.get("GCN_SKIPAG"))          # timing probe only
    MR = int(os.environ.get("GCN_MAXRANKS", str(MAXRANKS)))

    dt = mybir.dt
    ALU = mybir.AluOpType

    nc = bacc.Bacc("TRN2", target_bir_lowering=False, debug=False,
                   num_devices=NCORES)

    gidx_d = nc.dram_tensor("gidx", [128, total_tok // 16], dt.int16, kind="ExternalInput")
    xT_d = nc.dram_tensor("xT", [FIN, NPAD], dt.float32, kind="ExternalInput")
    W1_d = nc.dram_tensor("W1", [FIN, FHID], dt.float32, kind="ExternalInput")
    W2_d = nc.dram_tensor("W2", [FHID, FOUT], dt.float32, kind="ExternalInput")
    dis2_d = nc.dram_tensor("dis2c", [128, NW], dt.float32, kind="ExternalInput")
    dis_d = nc.dram_tensor("disc", [128, NW], dt.float32, kind="ExternalInput")
    Bstt_d = nc.dram_tensor("Bstt", [128, NW * FHID], dt.float32, kind="ExternalInput")
    b2b_d = nc.dram_tensor("b2b", [128, FOUT], dt.float32, kind="ExternalInput")
    ident_d = nc.dram_tensor("ident", [128, 128], dt.float32, kind="ExternalInput")
    out_d = nc.dram_tensor("out", [NPC, FOUT], dt.float32, kind="ExternalOutput")

    t1_local = nc.dram_tensor("t1_local", [NPC2, FHID], dt.float32)
    t1_full = nc.dram_tensor("t1_full", [NFULL, FHID], dt.float32, addr_space="Shared")
    t2_local = nc.dram_tensor("t2_local", [NPC2, FOUT], dt.float32)
    t2_full = nc.dram_tensor("t2_full", [NFULL, FOUT], dt.float32, addr_space="Shared")

    NWF = NW - 1  # 48 full windows; window 48 has 106 live rows

    with tile.TileContext(nc) as tc:
        with (
            tc.tile_pool(name="consts", bufs=1) as cp,
            tc.tile_pool(name="work", bufs=1) as wp,
            tc.tile_pool(name="psum", bufs=1, space="PSUM") as pp,
        ):
            ident_t = cp.tile([128, 128], dt.float32, tag="ident")
            nc.sync.dma_start(ident_t[:], ident_d[:, :])
            w1_t = cp.tile([128, 2, FHID], dt.float32, tag="w1")
            nc.sync.dma_start(w1_t[:, 0, :], W1_d[0:128, :])
            nc.sync.dma_start(w1_t[:, 1, :], W1_d[128:256, :])
            w2_t = cp.tile([FHID, FOUT], dt.float32, tag="w2")
            nc.sync.dma_start(w2_t[:], W2_d[:, :])
            dis2_t = cp.tile([128, NW], dt.float32, tag="dis2")
            nc.sync.dma_start(dis2_t[:], dis2_d[:, :])
            dis_t = cp.tile([128, NW], dt.float32, tag="dis")
            nc.sync.dma_start(dis_t[:], dis_d[:, :])
            if not B1ZERO:
                Bstt_t = cp.tile([128, NW * FHID], dt.float32, tag="Bstt")
                nc.sync.dma_start(Bstt_t[:], Bstt_d[:, :])
            b2b_t = cp.tile([128, FOUT], dt.float32, tag="b2b")
            nc.sync.dma_start(b2b_t[:], b2b_d[:, :])
            gidx_t = cp.tile([128, total_tok // 16], dt.int16, tag="gidx")
            nc.sync.dma_start(gidx_t[:], gidx_d[:, :])
            zrow = cp.tile([128, FHID], dt.float32, tag="zrow")
            nc.vector.memset(zrow[:], 0.0)

            # one shared gpsimd register per distinct gather count: avoids a
            # RegisterMove instruction (~55us here) per dma_gather
            counts = set()
            for w in range(NW):
                for nk in (int(KLOm[w]), int(KHIm[w])):
                    for k0 in range(0, nk, MR):
                        counts.add(min(MR, nk - k0) * 128)
            nidx_regs = {cnt: nc.gpsimd.to_reg(cnt) for cnt in sorted(counts)}

            dis2_bc = dis2_t[:].rearrange("p (w o) -> p w o", o=1) \
                               .broadcast_to([128, NW, FHID])
            dis_bc = dis_t[:].rearrange("p (w o) -> p w o", o=1) \
                             .broadcast_to([128, NW, FOUT])
            b2_bc = b2b_t[:].rearrange("(o p) f -> p o f", o=1) \
                            .broadcast_to([128, NW, FOUT])

            for _rep in range(REPEAT):
                # ---- phase B: t1_local = (dis*x) @ W1 ----
                with tc.tile_pool(name="phaseB", bufs=1) as pb:
                    xT_t = pb.tile([128, 2, NPAD], dt.float32, tag="xT")
                    nc.sync.dma_start(xT_t[:, 0, :], xT_d[0:128, :])
                    nc.sync.dma_start(xT_t[:, 1, :], xT_d[128:256, :])
                    evB = pb.tile([128, NWF, FHID], dt.float32, tag="evB")
                    evBt = pb.tile([128, FHID], dt.float32, tag="evBt")
                    psB = pp.tile([128, 8, FHID], dt.float32, tag="pB")
                    for w in range(NW):
                        sl = psB[:, w % 8, :]
                        for kc in range(2):
                            nc.tensor.matmul(
                                sl, xT_t[:, kc, w * 128:w * 128 + 128],
                                w1_t[:, kc, :], start=(kc == 0), stop=(kc == 1))
                        if w % 8 == 7:
                            nc.vector.tensor_copy(evB[:, w - 7:w + 1, :], psB[:])
                        if w == NW - 1:
                            nc.vector.tensor_copy(evBt[:], sl)
                    nc.sync.dma_start(
                        t1_local[0:NWF * 128, :]
                        .rearrange("(a p) f -> p a f", p=128), evB[:])
                    nc.sync.dma_start(t1_local[NWF * 128:NPC, :],
                                      evBt[0:NPC - NWF * 128, :])
                    nc.sync.dma_start(t1_local[NPC:NPC2, :], zrow[0:1, :])

                if SKIPAG:
                    nc.sync.dma_start(t1_full[0:NPC2, :], t1_local[:, :])
                else:
                    nc.gpsimd.collective_compute(
                        "AllGather", mybir.AluOpType.bypass,
                        replica_groups=[list(range(NCORES))],
                        ins=[t1_local[:, :]], outs=[t1_full[:, :]],
                    )

                if PHASES == "Bdump":
                    # debug: out <- first FOUT cols of t1_local
                    nc.sync.dma_start(out_d[:, :], t1_local[0:NPC, 0:FOUT])
                    continue

                if PHASES == "B":
                    ot = wp.tile([128, FOUT], dt.float32, tag="o")
                    nc.vector.memset(ot[:], 0.0)
                    for w in range(NW):
                        rows = min(128, NPC - w * 128)
                        nc.sync.dma_start(out_d[w * 128:w * 128 + rows, :],
                                          ot[0:rows, :])
                    continue

                def gather_window(tok, w, src_full, feat, woff_w, dk=0):
                    """Emit gathers for window w into tok at rank offset dk."""
                    klo, khi = int(KLOm[w]), int(KHIm[w])
                    base_lo = src_full[0:HALF, :]
                    base_hi = src_full[H2:NFULL, :]
                    segs = [(0, klo, base_lo), (klo, khi, base_hi)]
                    for seg0, nk, base in segs:
                        for k0 in range(0, nk, MR):
                            kn = min(MR, nk - k0)
                            c0 = (woff_w + (seg0 + k0) * 128) // 16
                            d0 = dk + seg0 + k0
                            if SKIPGATHER:
                                continue
                            nc.gpsimd.dma_gather(
                                tok[:, d0:d0 + kn, :], base,
                                gidx_t[:, c0:c0 + kn * 8],
                                num_idxs=kn * 128,
                                num_idxs_reg=nidx_regs[kn * 128],
                                elem_size=feat,
                                single_packet=(kn * 128 <= 1024))

                # ---- L1 pass 1: gather + reduce into red_all, one fused
                #      relu+scale over all windows ----
                with tc.tile_pool(name="L1", bufs=1) as l1:
                    tok = l1.tile([128, maxK, FHID], dt.float32, tag="tok1")
                    red_all = l1.tile([128, NW, FHID], dt.float32, tag="redA")
                    o1s_all = l1.tile([128, NW, FHID], dt.float32, tag="o1sa")
                    o1T = l1.tile([128, 4, FHID], dt.float32, tag="o1T")
                    ev1 = l1.tile([128, NWF, FOUT], dt.float32, tag="ev1")
                    ev1t = l1.tile([128, FOUT], dt.float32, tag="ev1t")
                    pT = pp.tile([128, 4, 512], dt.float32, tag="pT")  # slice per bank
                    p2 = pp.tile([128, 8, FOUT], dt.float32, tag="p2")
                    if SKIPGATHER:
                        nc.vector.memset(tok[:], 0.0)
                    if SKIPREDUCE:
                        nc.vector.memset(red_all[:], 0.0)
                    woff_w = 0
                    for w0 in range(0, NW, 2):
                        pair = [w0] if w0 + 1 >= NW else [w0, w0 + 1]
                        kp = int(K[w0])
                        for j, w in enumerate(pair):
                            gather_window(tok, w, t1_full, FHID,
                                          woff_w, j * kp)
                            woff_w += int(K[w]) * 128
                        if not SKIPREDUCE:
                            nc.vector.tensor_reduce(
                                red_all[:, w0:w0 + len(pair), :],
                                tok[:, 0:len(pair) * kp, :]
                                .rearrange("p (b k) f -> p b f k", b=len(pair)),
                                mybir.AxisListType.X, ALU.add)
                    # self-loop contribution: red += own t1 rows
                    own1 = l1.tile([128, NW, FHID], dt.float32, tag="own1")
                    nc.vector.memset(own1[:], 0.0)
                    nc.sync.dma_start(
                        own1[:, 0:NWF, :],
                        t1_local[0:NWF * 128, :]
                        .rearrange("(a p) f -> p a f", p=128))
                    nc.sync.dma_start(own1[0:NPC - NWF * 128, NWF, :],
                                      t1_local[NWF * 128:NPC, :])
                    nc.vector.scalar_tensor_tensor(
                        red_all[:], red_all[:], 0.0, own1[:],
                        ALU.bypass, ALU.add)
                    # o1s = relu(dis^2*red + dis*b1), batched over all windows
                    if B1ZERO:
                        if os.environ.get("GCN_NOBCAST"):
                            for w in range(NW):
                                nc.vector.tensor_scalar(
                                    o1s_all[:, w, :], red_all[:, w, :],
                                    dis2_t[:, w:w + 1], 0.0, ALU.mult, ALU.max)
                        else:
                            nc.vector.scalar_tensor_tensor(
                                o1s_all[:], red_all[:], 0.0, dis2_bc,
                                ALU.max, ALU.mult)
                    else:
                        nc.vector.scalar_tensor_tensor(
                            o1s_all[:], red_all[:], 0.0, dis2_bc,
                            ALU.bypass, ALU.mult)
                        nc.vector.scalar_tensor_tensor(
                            o1s_all[:], o1s_all[:], 0.0,
                            Bstt_t[:].rearrange("p (w f) -> p w f", w=NW),
                            ALU.bypass, ALU.add)
                        nc.vector.tensor_scalar(
                            o1s_all[:], o1s_all[:], 0.0, None, ALU.max)
                    # ---- L1 pass 2: transpose + @W2, batched ----
                    for w in range(NW):
                        nc.tensor.transpose(pT[:, w % 4, 0:FHID],
                                            o1s_all[:, w, :], ident_t[:])
                        if w % 4 == 3:
                            nc.vector.tensor_copy(o1T[:], pT[:, :, 0:FHID])
                        if w == NW - 1:
                            nc.vector.tensor_copy(o1T[:, 0, :], pT[:, 0, 0:FHID])
                        if w % 4 == 3 or w == NW - 1:
                            for w2 in range(w - (3 if w % 4 == 3 else 0), w + 1):
                                nc.tensor.matmul(p2[:, w2 % 8, :],
                                                 o1T[:, w2 % 4, :], w2_t[:],
                                                 start=True, stop=True)
                        if w % 8 == 7:
                            nc.vector.tensor_copy(ev1[:, w - 7:w + 1, :], p2[:])
                        if w == NW - 1:
                            nc.vector.tensor_copy(ev1t[:], p2[:, 0, :])
                    nc.sync.dma_start(
                        t2_local[0:NWF * 128, :]
                        .rearrange("(a p) f -> p a f", p=128), ev1[:])
                    nc.sync.dma_start(t2_local[NWF * 128:NPC, :],
                                      ev1t[0:NPC - NWF * 128, :])
                    nc.sync.dma_start(t2_local[NPC:NPC2, :], zrow[0:1, 0:FOUT])

                if PHASES == "B1dump":
                    nc.sync.dma_start(out_d[:, :], t2_local[0:NPC, :])
                    continue

                if PHASES == "B1":
                    ot = wp.tile([128, FOUT], dt.float32, tag="o")
                    nc.vector.memset(ot[:], 0.0)
                    for w in range(NW):
                        rows = min(128, NPC - w * 128)
                        nc.sync.dma_start(out_d[w * 128:w * 128 + rows, :],
                                          ot[0:rows, :])
                    continue

                if SKIPAG:
                    nc.sync.dma_start(t2_full[0:NPC2, :], t2_local[:, :])
                else:
                    nc.gpsimd.collective_compute(
                        "AllGather", mybir.AluOpType.bypass,
                        replica_groups=[list(range(NCORES))],
                        ins=[t2_local[:, :]], outs=[t2_full[:, :]],
                    )

                # ---- L2 windows ----
                with tc.tile_pool(name="L2", bufs=1) as l2:
                    tok2 = l2.tile([128, maxK, FOUT], dt.float32, tag="tok2")
                    red2_all = l2.tile([128, NW, FOUT], dt.float32, tag="red2A")
                    ev2_all = l2.tile([128, NW, FOUT], dt.float32, tag="ev2A")
                    if SKIPGATHER:
                        nc.vector.memset(tok2[:], 0.0)
                    if SKIPREDUCE:
                        nc.vector.memset(red2_all[:], 0.0)
                    woff_w = 0
                    for w0 in range(0, NW, 2):
                        pair = [w0] if w0 + 1 >= NW else [w0, w0 + 1]
                        kp = int(K[w0])
                        for j, w in enumerate(pair):
                            gather_window(tok2, w, t2_full, FOUT,
                                          woff_w, j * kp)
                            woff_w += int(K[w]) * 128
                        if not SKIPREDUCE:
                            nc.vector.tensor_reduce(
                                red2_all[:, w0:w0 + len(pair), :],
                                tok2[:, 0:len(pair) * kp, :]
                                .rearrange("p (b k) f -> p b f k", b=len(pair)),
                                mybir.AxisListType.X, ALU.add)
                    # self-loop contribution: red2 += own t2 rows
                    own2 = l2.tile([128, NW, FOUT], dt.float32, tag="own2")
                    nc.vector.memset(own2[:], 0.0)
                    nc.sync.dma_start(
                        own2[:, 0:NWF, :],
                        t2_local[0:NWF * 128, :]
                        .rearrange("(a p) f -> p a f", p=128))
                    nc.sync.dma_start(own2[0:NPC - NWF * 128, NWF, :],
                                      t2_local[NWF * 128:NPC, :])
                    nc.vector.scalar_tensor_tensor(
                        red2_all[:], red2_all[:], 0.0, own2[:],
                        ALU.bypass, ALU.add)
                    # out = dis*red2 (+ b2), batched over all windows
                    if os.environ.get("GCN_NOBCAST"):
                        for w in range(NW):
                            nc.vector.scalar_tensor_tensor(
                                ev2_all[:, w, :], red2_all[:, w, :],
                                dis_t[:, w:w + 1], b2b_t[:],
                                ALU.mult, ALU.add)
                    else:
                        nc.vector.scalar_tensor_tensor(
                            ev2_all[:], red2_all[:], 0.0, dis_bc,
                            ALU.bypass, ALU.mult)
                        if not B2ZERO:
                            nc.vector.scalar_tensor_tensor(
                                ev2_all[:], ev2_all[:], 0.0, b2_bc,
                                ALU.bypass, ALU.add)
                    nc.sync.dma_start(
                        out_d[0:NWF * 128, :]
                        .rearrange("(a p) f -> p a f", p=128),
                        ev2_all[:, 0:NWF, :])
                    nc.sync.dma_start(out_d[NWF * 128:NPC, :],
                                      ev2_all[0:NPC - NWF * 128, NWF, :])

    nc.compile()
    return nc


def kernel(x, edge_index, W1, b1, W2, b2):
    global LAST_RESULTS
    from concourse.bass_utils import run_bass_kernel_spmd

    in_maps, Kinfo, perms = _host_prep(x, edge_index, W1, b1, W2, b2)
    key = (Kinfo[0].tobytes(), Kinfo[1].tobytes(), Kinfo[2], Kinfo[3])
    if key not in _CACHE:
        _CACHE[key] = _build(Kinfo)
    nc = _CACHE[key]

    res = run_bass_kernel_spmd(nc, in_maps, list(range(NCORES)))
    LAST_RESULTS = res
    out = np.empty((N, FOUT), np.float32)
    for c in range(NCORES):
        out[c * NPC + perms[c]] = res.results[c]["out"]
    return out


# revision 23
# speedup vs baseline: 1.3482x; 1.0161x over previous
"""GCN encoder (2-layer) on 8 Trainium2 NeuronCores.

Math (per layer, matching the reference):
    out[d] = dis[d] * sum_{e: dst_e=d} dis[src_e] * h[src_e]  + b
with h = x @ W, dis = deg^-1/2 over src-with-self-loops. dis factors are
folded host-side: xT is pre-scaled by dis (layer-1 operand), layer-1's
output scaling uses dis^2 (post relu identity: dis*relu(z) = relu(dis*z)),
layer 2 applies dis at the end.

Design notes (this environment charges ~55us of fixed overhead per engine
instruction on top of data-proportional execution, so both instruction
count and gathered-token count are minimized):
  - dst nodes are assigned to [window, slot] positions sorted by in-degree
    (per core), so the per-window max rank K tracks the degree profile
    instead of the global max: padded gather tokens drop ~40%. Outputs are
    unpermuted host-side.
  - edges per dst-window (128 slots) are gathered in [rank, slot] order so
    token k*128+p is the k-th in-edge of window-slot p; one wide
    tensor_reduce over the rank axis aggregates a whole window pair.
  - per-window scale/bias ops are replaced by single whole-tensor
    scalar_tensor_tensor instructions using stride-0 broadcast APs.
  - gathers are issued as 1024-index single_packet instructions: the
    executor's per-token cost is ~12% lower than multi-packet mode and
    instruction count carries no measurable cost (single_packet=True
    hangs above ~1024 indices).
  - rank padding points at injected all-zero rows: every core ships 6251
    rows (row 6250 zeroed), so zero rows exist in both the lo ([0,32768))
    and hi ([17240,50008)) gather bases of the int16-index split. The two
    bases overlap: edges sourcing rows in the overlap are assigned to
    whichever stream balances the per-window rank counts.
Sharding: nodes row-sharded 6250/core, edges partitioned by dst core,
weights replicated, AllGather between layers.
"""
import os
import numpy as np

N, E = 50000, 1600000
FIN, FHID, FOUT = 256, 128, 64
NCORES = 8
NPC = N // NCORES          # 6250
NPC2 = NPC + 1             # 6251 rows shipped per core (last = zeros)
NFULL = NCORES * NPC2      # 50008
NW = (NPC + 127) // 128    # 49 windows
NPAD = NW * 128            # 6272
HALF = 32768               # lo gather base covers rows [0, HALF)
H2 = NFULL - HALF          # hi base covers rows [H2, NFULL); overlap rows
                           # [H2, HALF) may use either stream (flex)
ZLO = 6250                 # zero row inside lo base (core 0 pad row)
ZHI = 2 * NPC2 + NPC - H2  # core 2 pad row, hi-base-local index
MAXRANKS = 8               # ranks per gather instruction (1024 idxs);
                           # <=1024 idxs allows single_packet mode, which
                           # the executor runs ~12% faster per token

_CACHE = {}
LAST_RESULTS = None


def _host_prep(x, edge_index, W1, b1, W2, b2):
    x = np.asarray(x, dtype=np.float32)
    ei = np.asarray(edge_index)
    W1 = np.asarray(W1, dtype=np.float32)
    W2 = np.asarray(W2, dtype=np.float32)
    b1 = np.asarray(b1, dtype=np.float32)
    b2 = np.asarray(b2, dtype=np.float32)

    loops = np.arange(N, dtype=np.int64)
    src0 = np.concatenate([ei[0].astype(np.int64), loops])
    dst0 = np.concatenate([ei[1].astype(np.int64), loops])

    deg = np.bincount(src0, minlength=N).astype(np.float32)
    dis0 = np.power(deg, np.float32(-0.5), dtype=np.float32)
    dis0[deg == 0] = 0.0

    # Relabel nodes within each core by in-degree (descending) so the
    # per-window max rank K tracks the degree profile instead of the
    # global max: padded gather tokens drop ~40%. Everything downstream
    # (xT, t1, t2, gidx, dis tables) lives in position space; outputs are
    # unpermuted host-side.
    deg_in = np.bincount(dst0, minlength=N)
    srcE0 = ei[0].astype(np.int64)
    dstE0 = ei[1].astype(np.int64)

    def _mk_pos(keys):
        perms, pos_of = [], np.empty(N, np.int64)
        for c in range(NCORES):
            sl0 = slice(c * NPC, (c + 1) * NPC)
            oc = np.lexsort(tuple(k[sl0] for k in keys))
            perms.append(oc)
            pos_of[sl0][oc] = np.arange(NPC)
        return perms, pos_of

    # pass 1 (in-degree) fixes approximate source regions; pass 2 groups
    # windows by (degree, hi-count, lo-count) so the per-window maxima of
    # the lo/hi stream counts are tight, not just the max degree. A stale
    # key only costs padding, never correctness: the K schedule below is
    # recomputed exactly from the final positions.
    _, pos1 = _mk_pos([-deg_in])
    rP = (srcE0 // NPC) * NPC2 + pos1[srcE0]
    clsP = np.where(rP >= HALF, 0, np.where(rP < H2, 2, 1))
    lo_d = np.bincount(dstE0[clsP == 2], minlength=N)
    hi_d = np.bincount(dstE0[clsP == 0], minlength=N)
    D_d = np.bincount(dstE0, minlength=N)
    perms, pos_of = _mk_pos([-lo_d, -hi_d, -D_d])
    # Self-loops are excluded from the gather pipeline: the kernel adds each
    # node's own t1/t2 row directly (an aligned DMA load + one add), which
    # removes 6250 real tokens per core per layer and lowers every window's
    # max rank by one. dis still uses the with-loops degree (reference).
    src = (srcE0 // NPC) * NPC + pos_of[srcE0]
    dst = (dstE0 // NPC) * NPC + pos_of[dstE0]
    dis = np.concatenate([dis0[c * NPC:(c + 1) * NPC][perms[c]]
                          for c in range(NCORES)])

    # padded gather row of each source node
    r_all = (src // NPC) * NPC2 + (src % NPC)

    order = np.argsort(dst, kind="stable")
    r_s, dst_s = r_all[order], dst[order]
    cb = np.searchsorted(dst_s, np.arange(NCORES + 1) * NPC)

    # Stream classes: rows < H2 must use the lo base, rows >= HALF must
    # use the hi base, rows in [H2, HALF) may use either. Per (window,
    # slot), flexible edges are assigned to balance the two streams, so
    # the window rank count K tracks the max in-degree instead of
    # max(lo-degree) + max(hi-degree).
    wq_all = np.arange(NPC) // 128
    DegM = np.zeros(NW, np.int64)   # max slot degree per window
    LoM = np.zeros(NW, np.int64)    # max slot lo_only count
    HiM = np.zeros(NW, np.int64)    # max slot hi_only count
    pre = []
    for c in range(NCORES):
        sl = slice(cb[c], cb[c + 1])
        r_c = r_s[sl]
        d_c = dst_s[sl] - c * NPC
        # class: 0 = hi_only, 1 = flex, 2 = lo_only (order = hi fill pref)
        cls = np.where(r_c >= HALF, 0, np.where(r_c < H2, 2, 1))
        deg_q = np.bincount(d_c, minlength=NPC)
        hi_q = np.bincount(d_c[cls == 0], minlength=NPC)
        lo_q = np.bincount(d_c[cls == 2], minlength=NPC)
        np.maximum.at(DegM, wq_all, deg_q)
        np.maximum.at(LoM, wq_all, lo_q)
        np.maximum.at(HiM, wq_all, hi_q)
        pre.append((r_c, d_c, cls, deg_q, hi_q))
    Kw = np.maximum(DegM, LoM + HiM)
    KHIm = HiM.copy()               # hi segment = exactly HiM ranks
    KLOm = Kw - HiM                 # lo segment absorbs the slack

    percore = []
    for c in range(NCORES):
        r_c, d_c, cls, deg_q, hi_q = pre[c]
        # per-slot hi-stream target: all hi_only plus enough flex to keep
        # the lo count within KLOm
        h_q = np.maximum(hi_q, deg_q - KLOm[wq_all])
        key = d_c * 4 + cls
        o2 = np.argsort(key, kind="stable")
        d_o, r_o = d_c[o2], r_c[o2]
        grp = np.searchsorted(d_o, d_o, side="left")
        i_in = np.arange(len(d_o)) - grp     # index within slot
        hq_o = h_q[d_o]
        s_o = (i_in < hq_o).astype(np.int64)  # 1 = hi stream
        rank = np.where(s_o == 1, i_in, i_in - hq_o)
        w_o, p_o = d_o // 128, d_o % 128
        percore.append((w_o, p_o, s_o, rank, r_o))
    # pad window pairs (2w, 2w+1) to equal total ranks so one 4D-AP
    # tensor_reduce can aggregate both windows at once
    Kt = KLOm + KHIm
    for i in range(0, NW - 1, 2):
        kp = max(Kt[i], Kt[i + 1])
        KHIm[i] += kp - Kt[i]
        KHIm[i + 1] += kp - Kt[i + 1]
    K = KLOm + KHIm
    # flat token-position offsets: window w = [lo ranks][hi ranks]
    woff = np.zeros(NW + 1, np.int64)
    woff[1:] = np.cumsum(K) * 128
    total_tok = int(woff[-1])

    in_maps = []
    for c in range(NCORES):
        w_o, p_o, s_o, rank, r_o = percore[c]
        gidx = np.empty(total_tok, np.int16)
        for w in range(NW):
            gidx[woff[w]:woff[w] + KLOm[w] * 128] = ZLO
            gidx[woff[w] + KLOm[w] * 128:woff[w + 1]] = ZHI
        pos = woff[w_o] + (rank + np.where(s_o == 1, KLOm[w_o], 0)) * 128 + p_o
        gidx[pos] = np.where(s_o == 1, r_o - H2, r_o).astype(np.int16)
        gidx_t = np.tile(gidx.reshape(-1, 16).T, (8, 1))  # [128, total_tok//16]

        dis_l = dis[c * NPC:(c + 1) * NPC]     # position space
        dis_pad = np.zeros(NPAD, np.float32)
        dis_pad[:NPC] = dis_l
        dis_col = np.ascontiguousarray(dis_pad.reshape(NW, 128).T)  # [128, NW]
        dis2_col = dis_col * dis_col
        # Bstt[p, w*128+f] = dis[w*128+p] * b1[f]
        Bstt = (dis_col.T[:, :, None] * b1[None, None, :]).transpose(1, 0, 2)
        Bstt = np.ascontiguousarray(Bstt.reshape(128, NW * FHID))

        xT = np.zeros((FIN, NPAD), np.float32)
        xT[:, :NPC] = (x[c * NPC + perms[c]] * dis_l[:, None]).T

        in_maps.append({
            "gidx": np.ascontiguousarray(gidx_t),
            "xT": xT,
            "W1": W1, "W2": W2,
            "dis2c": dis2_col, "disc": dis_col,
            "Bstt": Bstt,
            "b2b": np.tile(b2, (128, 1)),
            "ident": np.eye(128, dtype=np.float32),
        })
    return in_maps, (KLOm, KHIm, bool(not b1.any()), bool(not b2.any())), perms


def _build(Kinfo):
    import concourse.bacc as bacc
    import concourse.mybir as mybir
    import concourse.tile as tile

    KLOm, KHIm, B1ZERO, B2ZERO = Kinfo
    K = KLOm + KHIm
    maxK = max(int(K[i]) * (1 if i + 1 >= NW else 2)
               for i in range(0, NW, 2))
    total_tok = int(K.sum()) * 128

    PHASES = os.environ.get("GCN_PHASES", "full")
    REPEAT = int(os.environ.get("GCN_REPEAT", "1"))
    SKIPGATHER = bool(os.environ.get("GCN_SKIPGATHER"))  # timing probe only
    SKIPREDUCE = bool(os.environ.get("GCN_SKIPREDUCE"))  # timing probe only
    SKIPAG = bool(os.environ.get("GCN_SKIPAG"))          # timing probe only
    MR = int(os.environ.get("GCN_MAXRANKS", str(MAXRANKS)))

    dt = mybir.dt
    ALU = mybir.AluOpType

    nc = bacc.Bacc("TRN2", target_bir_lowering=False, debug=False,
                   num_devices=NCORES)

    gidx_d = nc.dram_tensor("gidx", [128, total_tok // 16], dt.int16, kind="ExternalInput")
    xT_d = nc.dram_tensor("xT", [FIN, NPAD], dt.float32, kind="ExternalInput")
    W1_d = nc.dram_tensor("W1", [FIN, FHID], dt.float32, kind="ExternalInput")
    W2_d = nc.dram_tensor("W2", [FHID, FOUT], dt.float32, kind="ExternalInput")
    dis2_d = nc.dram_tensor("dis2c", [128, NW], dt.float32, kind="ExternalInput")
    dis_d = nc.dram_tensor("disc", [128, NW], dt.float32, kind="ExternalInput")
    Bstt_d = nc.dram_tensor("Bstt", [128, NW * FHID], dt.float32, kind="ExternalInput")
    b2b_d = nc.dram_tensor("b2b", [128, FOUT], dt.float32, kind="ExternalInput")
    ident_d = nc.dram_tensor("ident", [128, 128], dt.float32, kind="ExternalInput")
    out_d = nc.dram_tensor("out", [NPC, FOUT], dt.float32, kind="ExternalOutput")

    # double-buffered across repeats: repeat r+1's phase B / AllGather can
    # overlap repeat r's L2 gathers instead of serializing on a WAR hazard
    t1_locals = [nc.dram_tensor(f"t1_local{i}", [NPC2, FHID], dt.float32)
                 for i in range(2)]
    t1_fulls = [nc.dram_tensor(f"t1_full{i}", [NFULL, FHID], dt.float32,
                               addr_space="Shared") for i in range(2)]
    t2_locals = [nc.dram_tensor(f"t2_local{i}", [NPC2, FOUT], dt.float32)
                 for i in range(2)]
    t2_fulls = [nc.dram_tensor(f"t2_full{i}", [NFULL, FOUT], dt.float32,
                               addr_space="Shared") for i in range(2)]

    NWF = NW - 1  # 48 full windows; window 48 has 106 live rows

    with tile.TileContext(nc) as tc:
        with (
            tc.tile_pool(name="consts", bufs=1) as cp,
            tc.tile_pool(name="work", bufs=1) as wp,
            tc.tile_pool(name="psum", bufs=1, space="PSUM") as pp,
        ):
            ident_t = cp.tile([128, 128], dt.float32, tag="ident")
            nc.sync.dma_start(ident_t[:], ident_d[:, :])
            w1_t = cp.tile([128, 2, FHID], dt.float32, tag="w1")
            nc.sync.dma_start(w1_t[:, 0, :], W1_d[0:128, :])
            nc.sync.dma_start(w1_t[:, 1, :], W1_d[128:256, :])
            w2_t = cp.tile([FHID, FOUT], dt.float32, tag="w2")
            nc.sync.dma_start(w2_t[:], W2_d[:, :])
            dis2_t = cp.tile([128, NW], dt.float32, tag="dis2")
            nc.sync.dma_start(dis2_t[:], dis2_d[:, :])
            dis_t = cp.tile([128, NW], dt.float32, tag="dis")
            nc.sync.dma_start(dis_t[:], dis_d[:, :])
            if not B1ZERO:
                Bstt_t = cp.tile([128, NW * FHID], dt.float32, tag="Bstt")
                nc.sync.dma_start(Bstt_t[:], Bstt_d[:, :])
            b2b_t = cp.tile([128, FOUT], dt.float32, tag="b2b")
            nc.sync.dma_start(b2b_t[:], b2b_d[:, :])
            gidx_t = cp.tile([128, total_tok // 16], dt.int16, tag="gidx")
            nc.sync.dma_start(gidx_t[:], gidx_d[:, :])
            zrow = cp.tile([128, FHID], dt.float32, tag="zrow")
            nc.vector.memset(zrow[:], 0.0)

            # one shared gpsimd register per distinct gather count: avoids a
            # RegisterMove instruction (~55us here) per dma_gather
            counts = set()
            for w in range(NW):
                for nk in (int(KLOm[w]), int(KHIm[w])):
                    for k0 in range(0, nk, MR):
                        counts.add(min(MR, nk - k0) * 128)
            nidx_regs = {cnt: nc.gpsimd.to_reg(cnt) for cnt in sorted(counts)}

            dis2_bc = dis2_t[:].rearrange("p (w o) -> p w o", o=1) \
                               .broadcast_to([128, NW, FHID])
            dis_bc = dis_t[:].rearrange("p (w o) -> p w o", o=1) \
                             .broadcast_to([128, NW, FOUT])
            b2_bc = b2b_t[:].rearrange("(o p) f -> p o f", o=1) \
                            .broadcast_to([128, NW, FOUT])

            for _rep in range(REPEAT):
                t1_local, t1_full = t1_locals[_rep % 2], t1_fulls[_rep % 2]
                t2_local, t2_full = t2_locals[_rep % 2], t2_fulls[_rep % 2]
                # ---- phase B: t1_local = (dis*x) @ W1 ----
                with tc.tile_pool(name="phaseB", bufs=1) as pb:
                    xT_t = pb.tile([128, 2, NPAD], dt.float32, tag="xT")
                    nc.sync.dma_start(xT_t[:, 0, :], xT_d[0:128, :])
                    nc.sync.dma_start(xT_t[:, 1, :], xT_d[128:256, :])
                    evB = pb.tile([128, NWF, FHID], dt.float32, tag="evB")
                    evBt = pb.tile([128, FHID], dt.float32, tag="evBt")
                    psB = pp.tile([128, 8, FHID], dt.float32, tag="pB")
                    for w in range(NW):
                        sl = psB[:, w % 8, :]
                        for kc in range(2):
                            nc.tensor.matmul(
                                sl, xT_t[:, kc, w * 128:w * 128 + 128],
                                w1_t[:, kc, :], start=(kc == 0), stop=(kc == 1))
                        if w % 8 == 7:
                            nc.vector.tensor_copy(evB[:, w - 7:w + 1, :], psB[:])
                        if w == NW - 1:
                            nc.vector.tensor_copy(evBt[:], sl)
                    nc.sync.dma_start(
                        t1_local[0:NWF * 128, :]
                        .rearrange("(a p) f -> p a f", p=128), evB[:])
                    nc.sync.dma_start(t1_local[NWF * 128:NPC, :],
                                      evBt[0:NPC - NWF * 128, :])
                    nc.sync.dma_start(t1_local[NPC:NPC2, :], zrow[0:1, :])

                if SKIPAG:
                    nc.sync.dma_start(t1_full[0:NPC2, :], t1_local[:, :])
                else:
                    nc.gpsimd.collective_compute(
                        "AllGather", mybir.AluOpType.bypass,
                        replica_groups=[list(range(NCORES))],
                        ins=[t1_local[:, :]], outs=[t1_full[:, :]],
                    )

                if PHASES == "Bdump":
                    # debug: out <- first FOUT cols of t1_local
                    nc.sync.dma_start(out_d[:, :], t1_local[0:NPC, 0:FOUT])
                    continue

                if PHASES == "B":
                    ot = wp.tile([128, FOUT], dt.float32, tag="o")
                    nc.vector.memset(ot[:], 0.0)
                    for w in range(NW):
                        rows = min(128, NPC - w * 128)
                        nc.sync.dma_start(out_d[w * 128:w * 128 + rows, :],
                                          ot[0:rows, :])
                    continue

                def gather_window(tok, w, src_full, feat, woff_w, dk=0):
                    """Emit gathers for window w into tok at rank offset dk."""
                    klo, khi = int(KLOm[w]), int(KHIm[w])
                    base_lo = src_full[0:HALF, :]
                    base_hi = src_full[H2:NFULL, :]
                    segs = [(0, klo, base_lo), (klo, khi, base_hi)]
                    for seg0, nk, base in segs:
                        for k0 in range(0, nk, MR):
                            kn = min(MR, nk - k0)
                            c0 = (woff_w + (seg0 + k0) * 128) // 16
                            d0 = dk + seg0 + k0
                            if SKIPGATHER:
                                continue
                            nc.gpsimd.dma_gather(
                                tok[:, d0:d0 + kn, :], base,
                                gidx_t[:, c0:c0 + kn * 8],
                                num_idxs=kn * 128,
                                num_idxs_reg=nidx_regs[kn * 128],
                                elem_size=feat,
                                single_packet=(kn * 128 <= 1024))

                # ---- L1 pass 1: gather + reduce into red_all, one fused
                #      relu+scale over all windows ----
                with tc.tile_pool(name="L1", bufs=1) as l1:
                    tok = l1.tile([128, maxK, FHID], dt.float32, tag="tok1")
                    red_all = l1.tile([128, NW, FHID], dt.float32, tag="redA")
                    o1s_all = l1.tile([128, NW, FHID], dt.float32, tag="o1sa")
                    o1T = l1.tile([128, 4, FHID], dt.float32, tag="o1T")
                    ev1 = l1.tile([128, NWF, FOUT], dt.float32, tag="ev1")
                    ev1t = l1.tile([128, FOUT], dt.float32, tag="ev1t")
                    pT = pp.tile([128, 4, 512], dt.float32, tag="pT")  # slice per bank
                    p2 = pp.tile([128, 8, FOUT], dt.float32, tag="p2")
                    if SKIPGATHER:
                        nc.vector.memset(tok[:], 0.0)
                    if SKIPREDUCE:
                        nc.vector.memset(red_all[:], 0.0)
                    woff_w = 0
                    for w0 in range(0, NW, 2):
                        pair = [w0] if w0 + 1 >= NW else [w0, w0 + 1]
                        kp = int(K[w0])
                        for j, w in enumerate(pair):
                            gather_window(tok, w, t1_full, FHID,
                                          woff_w, j * kp)
                            woff_w += int(K[w]) * 128
                        if not SKIPREDUCE:
                            nc.vector.tensor_reduce(
                                red_all[:, w0:w0 + len(pair), :],
                                tok[:, 0:len(pair) * kp, :]
                                .rearrange("p (b k) f -> p b f k", b=len(pair)),
                                mybir.AxisListType.X, ALU.add)
                    # self-loop contribution: red += own t1 rows
                    own1 = l1.tile([128, NW, FHID], dt.float32, tag="own1")
                    nc.vector.memset(own1[:], 0.0)
                    nc.sync.dma_start(
                        own1[:, 0:NWF, :],
                        t1_local[0:NWF * 128, :]
                        .rearrange("(a p) f -> p a f", p=128))
                    nc.sync.dma_start(own1[0:NPC - NWF * 128, NWF, :],
                                      t1_local[NWF * 128:NPC, :])
                    nc.vector.scalar_tensor_tensor(
                        red_all[:], red_all[:], 0.0, own1[:],
                        ALU.bypass, ALU.add)
                    # o1s = relu(dis^2*red + dis*b1), batched over all windows
                    if B1ZERO:
                        if os.environ.get("GCN_NOBCAST"):
                            for w in range(NW):
                                nc.vector.tensor_scalar(
                                    o1s_all[:, w, :], red_all[:, w, :],
                                    dis2_t[:, w:w + 1], 0.0, ALU.mult, ALU.max)
                        else:
                            nc.vector.scalar_tensor_tensor(
                                o1s_all[:], red_all[:], 0.0, dis2_bc,
                                ALU.max, ALU.mult)
                    else:
                        nc.vector.scalar_tensor_tensor(
                            o1s_all[:], red_all[:], 0.0, dis2_bc,
                            ALU.bypass, ALU.mult)
                        nc.vector.scalar_tensor_tensor(
                            o1s_all[:], o1s_all[:], 0.0,
                            Bstt_t[:].rearrange("p (w f) -> p w f", w=NW),
                            ALU.bypass, ALU.add)
                        nc.vector.tensor_scalar(
                            o1s_all[:], o1s_all[:], 0.0, None, ALU.max)
                    # ---- L1 pass 2: transpose + @W2, batched ----
                    for w in range(NW):
                        nc.tensor.transpose(pT[:, w % 4, 0:FHID],
                                            o1s_all[:, w, :], ident_t[:])
                        if w % 4 == 3:
                            nc.vector.tensor_copy(o1T[:], pT[:, :, 0:FHID])
                        if w == NW - 1:
                            nc.vector.tensor_copy(o1T[:, 0, :], pT[:, 0, 0:FHID])
                        if w % 4 == 3 or w == NW - 1:
                            for w2 in range(w - (3 if w % 4 == 3 else 0), w + 1):
                                nc.tensor.matmul(p2[:, w2 % 8, :],
                                                 o1T[:, w2 % 4, :], w2_t[:],
                                                 start=True, stop=True)
                        if w % 8 == 7:
                            nc.vector.tensor_copy(ev1[:, w - 7:w + 1, :], p2[:])
                        if w == NW - 1:
                            nc.vector.tensor_copy(ev1t[:], p2[:, 0, :])
                    nc.sync.dma_start(
                        t2_local[0:NWF * 128, :]
                        .rearrange("(a p) f -> p a f", p=128), ev1[:])
                    nc.sync.dma_start(t2_local[NWF * 128:NPC, :],
                                      ev1t[0:NPC - NWF * 128, :])
                    nc.sync.dma_start(t2_local[NPC:NPC2, :], zrow[0:1, 0:FOUT])

                if PHASES == "B1dump":
                    nc.sync.dma_start(out_d[:, :], t2_local[0:NPC, :])
                    continue

                if PHASES == "B1":
                    ot = wp.tile([128, FOUT], dt.float32, tag="o")
                    nc.vector.memset(ot[:], 0.0)
                    for w in range(NW):
                        rows = min(128, NPC - w * 128)
                        nc.sync.dma_start(out_d[w * 128:w * 128 + rows, :],
                                          ot[0:rows, :])
                    continue

                if SKIPAG:
                    nc.sync.dma_start(t2_full[0:NPC2, :], t2_local[:, :])
                else:
                    nc.gpsimd.collective_compute(
                        "AllGather", mybir.AluOpType.bypass,
                        replica_groups=[list(range(NCORES))],
                        ins=[t2_local[:, :]], outs=[t2_full[:, :]],
                    )

                # ---- L2 windows ----
                with tc.tile_pool(name="L2", bufs=1) as l2:
                    tok2 = l2.tile([128, maxK, FOUT], dt.float32, tag="tok2")
                    red2_all = l2.tile([128, NW, FOUT], dt.float32, tag="red2A")
                    ev2_all = l2.tile([128, NW, FOUT], dt.float32, tag="ev2A")
                    if SKIPGATHER:
                        nc.vector.memset(tok2[:], 0.0)
                    if SKIPREDUCE:
                        nc.vector.memset(red2_all[:], 0.0)
                    woff_w = 0
                    for w0 in range(0, NW, 2):
                        pair = [w0] if w0 + 1 >= NW else [w0, w0 + 1]
                        kp = int(K[w0])
                        for j, w in enumerate(pair):
                            gather_window(tok2, w, t2_full, FOUT,
                                          woff_w, j * kp)
                            woff_w += int(K[w]) * 128
                        if not SKIPREDUCE:
                            nc.vector.tensor_reduce(
                                red2_all[:, w0:w0 + len(pair), :],
                                tok2[:, 0:len(pair) * kp, :]
                                .rearrange("p (b k) f -> p b f k", b=len(pair)),
                                mybir.AxisListType.X, ALU.add)
                    # self-loop contribution: red2 += own t2 rows
                    own2 = l2.tile([128, NW, FOUT], dt.float32, tag="own2")
                    nc.vector.memset(own2[:], 0.0)
                    nc.sync.dma_start(
                        own2[:, 0:NWF, :],
                        t2_local[0:NWF * 128, :]
                        .rearrange("(a p) f -> p a f", p=128))
                    nc.sync.dma_start(own2[0:NPC - NWF * 128, NWF, :],
                                      t2_local[NWF * 128:NPC, :])
                    nc.vector.scalar_tensor_tensor(
                        red2_all[:], red2_all[:], 0.0, own2[:],
                        ALU.bypass, ALU.add)
                    # out = dis*red2 (+ b2), batched over all windows
                    if os.environ.get("GCN_NOBCAST"):
                        for w in range(NW):
                            nc.vector.scalar_tensor_tensor(
                                ev2_all[:, w, :], red2_all[:, w, :],
                                dis_t[:, w:w + 1], b2b_t[:],
                                ALU.mult, ALU.add)
                    else:
                        nc.vector.scalar_tensor_tensor(
                            ev2_all[:], red2_all[:], 0.0, dis_bc,
                            ALU.bypass, ALU.mult)
                        if not B2ZERO:
                            nc.vector.scalar_tensor_tensor(
                                ev2_all[:], ev2_all[:], 0.0, b2_bc,
                                ALU.bypass, ALU.add)
                    nc.sync.dma_start(
                        out_d[0:NWF * 128, :]
                        .rearrange("(a p) f -> p a f", p=128),
                        ev2_all[:, 0:NWF, :])
                    nc.sync.dma_start(out_d[NWF * 128:NPC, :],
                                      ev2_all[0:NPC - NWF * 128, NWF, :])

    nc.compile()
    return nc


def kernel(x, edge_index, W1, b1, W2, b2):
    global LAST_RESULTS
    from concourse.bass_utils import run_bass_kernel_spmd

    in_maps, Kinfo, perms = _host_prep(x, edge_index, W1, b1, W2, b2)
    key = (Kinfo[0].tobytes(), Kinfo[1].tobytes(), Kinfo[2], Kinfo[3])
    if key not in _CACHE:
        _CACHE[key] = _build(Kinfo)
    nc = _CACHE[key]

    res = run_bass_kernel_spmd(nc, in_maps, list(range(NCORES)))
    LAST_RESULTS = res
    out = np.empty((N, FOUT), np.float32)
    for c in range(NCORES):
        out[c * NPC + perms[c]] = res.results[c]["out"]
    return out


# revision 29
# speedup vs baseline: 1.9688x; 1.4604x over previous
"""GCN encoder (2-layer) on 8 Trainium2 NeuronCores.

Math (per layer, matching the reference):
    out[d] = dis[d] * sum_{e: dst_e=d} dis[src_e] * h[src_e]  + b
with h = x @ W, dis = deg^-1/2 over src-with-self-loops. dis factors are
folded host-side: xT is pre-scaled by dis (layer-1 operand), layer-1's
output scaling uses dis^2 (post relu identity: dis*relu(z) = relu(dis*z)),
layer 2 applies dis at the end.

Design notes (this environment charges ~55us of fixed overhead per engine
instruction on top of data-proportional execution, so both instruction
count and gathered-token count are minimized):
  - dst nodes are assigned to [window, slot] positions sorted by in-degree
    (per core), so the per-window max rank K tracks the degree profile
    instead of the global max: padded gather tokens drop ~40%. Outputs are
    unpermuted host-side.
  - edges per dst-window (128 slots) are gathered in [rank, slot] order so
    token k*128+p is the k-th in-edge of window-slot p; one wide
    tensor_reduce over the rank axis aggregates a whole window pair.
  - per-window scale/bias ops are replaced by single whole-tensor
    scalar_tensor_tensor instructions using stride-0 broadcast APs.
  - gathers are issued as 1024-index single_packet instructions: the
    executor's per-token cost is ~12% lower than multi-packet mode and
    instruction count carries no measurable cost (single_packet=True
    hangs above ~1024 indices).
  - rank padding points at injected all-zero rows: every core ships 6251
    rows (row 6250 zeroed), so zero rows exist in both the lo ([0,32768))
    and hi ([17240,50008)) gather bases of the int16-index split. The two
    bases overlap: edges sourcing rows in the overlap are assigned to
    whichever stream balances the per-window rank counts.
Sharding: nodes row-sharded 6250/core, edges partitioned by dst core,
weights replicated, AllGather between layers.
"""
import os
import numpy as np

N, E = 50000, 1600000
FIN, FHID, FOUT = 256, 128, 64
NCORES = 8
NPC = N // NCORES          # 6250
NPC2 = NPC + 1             # 6251 rows shipped per core (last = zeros)
NFULL = NCORES * NPC2      # 50008
NW = (NPC + 127) // 128    # 49 windows
NPAD = NW * 128            # 6272
HALF = 32768               # lo gather base covers rows [0, HALF)
H2 = NFULL - HALF          # hi base covers rows [H2, NFULL); overlap rows
                           # [H2, HALF) may use either stream (flex)
ZLO = 6250                 # zero row inside lo base (core 0 pad row)
ZHI = 2 * NPC2 + NPC - H2  # core 2 pad row, hi-base-local index
MAXRANKS = 8               # ranks per gather instruction (1024 idxs);
                           # <=1024 idxs allows single_packet mode, which
                           # the executor runs ~12% faster per token

_CACHE = {}
LAST_RESULTS = None


def _host_prep(x, edge_index, W1, b1, W2, b2):
    x = np.asarray(x, dtype=np.float32)
    ei = np.asarray(edge_index)
    W1 = np.asarray(W1, dtype=np.float32)
    W2 = np.asarray(W2, dtype=np.float32)
    b1 = np.asarray(b1, dtype=np.float32)
    b2 = np.asarray(b2, dtype=np.float32)

    loops = np.arange(N, dtype=np.int64)
    src0 = np.concatenate([ei[0].astype(np.int64), loops])
    dst0 = np.concatenate([ei[1].astype(np.int64), loops])

    deg = np.bincount(src0, minlength=N).astype(np.float32)
    dis0 = np.power(deg, np.float32(-0.5), dtype=np.float32)
    dis0[deg == 0] = 0.0

    # Relabel nodes within each core by in-degree (descending) so the
    # per-window max rank K tracks the degree profile instead of the
    # global max: padded gather tokens drop ~40%. Everything downstream
    # (xT, t1, t2, gidx, dis tables) lives in position space; outputs are
    # unpermuted host-side.
    deg_in = np.bincount(dst0, minlength=N)
    srcE0 = ei[0].astype(np.int64)
    dstE0 = ei[1].astype(np.int64)

    def _mk_pos(keys):
        perms, pos_of = [], np.empty(N, np.int64)
        for c in range(NCORES):
            sl0 = slice(c * NPC, (c + 1) * NPC)
            oc = np.lexsort(tuple(k[sl0] for k in keys))
            perms.append(oc)
            pos_of[sl0][oc] = np.arange(NPC)
        return perms, pos_of

    # pass 1 (in-degree) fixes approximate source regions; pass 2 groups
    # windows by (degree, hi-count, lo-count) so the per-window maxima of
    # the lo/hi stream counts are tight, not just the max degree. A stale
    # key only costs padding, never correctness: the K schedule below is
    # recomputed exactly from the final positions.
    _, pos1 = _mk_pos([-deg_in])
    rP = (srcE0 // NPC) * NPC2 + pos1[srcE0]
    clsP = np.where(rP >= HALF, 0, np.where(rP < H2, 2, 1))
    lo_d = np.bincount(dstE0[clsP == 2], minlength=N)
    hi_d = np.bincount(dstE0[clsP == 0], minlength=N)
    D_d = np.bincount(dstE0, minlength=N)
    perms, pos_of = _mk_pos([-lo_d, -hi_d, -D_d])
    # Self-loops are excluded from the gather pipeline: the kernel adds each
    # node's own t1/t2 row directly (an aligned DMA load + one add), which
    # removes 6250 real tokens per core per layer and lowers every window's
    # max rank by one. dis still uses the with-loops degree (reference).
    src = (srcE0 // NPC) * NPC + pos_of[srcE0]
    dst = (dstE0 // NPC) * NPC + pos_of[dstE0]
    dis = np.concatenate([dis0[c * NPC:(c + 1) * NPC][perms[c]]
                          for c in range(NCORES)])

    # padded gather row of each source node
    r_all = (src // NPC) * NPC2 + (src % NPC)

    order = np.argsort(dst, kind="stable")
    r_s, dst_s = r_all[order], dst[order]
    cb = np.searchsorted(dst_s, np.arange(NCORES + 1) * NPC)

    # Stream classes: rows < H2 must use the lo base, rows >= HALF must
    # use the hi base, rows in [H2, HALF) may use either. Per (window,
    # slot), flexible edges are assigned to balance the two streams, so
    # the window rank count K tracks the max in-degree instead of
    # max(lo-degree) + max(hi-degree).
    wq_all = np.arange(NPC) // 128
    DegM = np.zeros(NW, np.int64)   # max slot degree per window
    LoM = np.zeros(NW, np.int64)    # max slot lo_only count
    HiM = np.zeros(NW, np.int64)    # max slot hi_only count
    pre = []
    for c in range(NCORES):
        sl = slice(cb[c], cb[c + 1])
        r_c = r_s[sl]
        d_c = dst_s[sl] - c * NPC
        # class: 0 = hi_only, 1 = flex, 2 = lo_only (order = hi fill pref)
        cls = np.where(r_c >= HALF, 0, np.where(r_c < H2, 2, 1))
        deg_q = np.bincount(d_c, minlength=NPC)
        hi_q = np.bincount(d_c[cls == 0], minlength=NPC)
        lo_q = np.bincount(d_c[cls == 2], minlength=NPC)
        np.maximum.at(DegM, wq_all, deg_q)
        np.maximum.at(LoM, wq_all, lo_q)
        np.maximum.at(HiM, wq_all, hi_q)
        pre.append((r_c, d_c, cls, deg_q, hi_q))
    Kw = np.maximum(DegM, LoM + HiM)
    KHIm = HiM.copy()               # hi segment = exactly HiM ranks
    KLOm = Kw - HiM                 # lo segment absorbs the slack

    percore = []
    for c in range(NCORES):
        r_c, d_c, cls, deg_q, hi_q = pre[c]
        # per-slot hi-stream target: all hi_only plus enough flex to keep
        # the lo count within KLOm
        h_q = np.maximum(hi_q, deg_q - KLOm[wq_all])
        key = d_c * 4 + cls
        o2 = np.argsort(key, kind="stable")
        d_o, r_o = d_c[o2], r_c[o2]
        grp = np.searchsorted(d_o, d_o, side="left")
        i_in = np.arange(len(d_o)) - grp     # index within slot
        hq_o = h_q[d_o]
        s_o = (i_in < hq_o).astype(np.int64)  # 1 = hi stream
        rank = np.where(s_o == 1, i_in, i_in - hq_o)
        w_o, p_o = d_o // 128, d_o % 128
        percore.append((w_o, p_o, s_o, rank, r_o))
    # one tensor_reduce per window (reduce instructions are free in this
    # executor), so no pair padding is needed
    K = KLOm + KHIm
    # flat token-position offsets: window w = [lo ranks][hi ranks]
    woff = np.zeros(NW + 1, np.int64)
    woff[1:] = np.cumsum(K) * 128
    total_tok = int(woff[-1])

    in_maps = []
    for c in range(NCORES):
        w_o, p_o, s_o, rank, r_o = percore[c]
        gidx = np.empty(total_tok, np.int16)
        for w in range(NW):
            gidx[woff[w]:woff[w] + KLOm[w] * 128] = ZLO
            gidx[woff[w] + KLOm[w] * 128:woff[w + 1]] = ZHI
        pos = woff[w_o] + (rank + np.where(s_o == 1, KLOm[w_o], 0)) * 128 + p_o
        gidx[pos] = np.where(s_o == 1, r_o - H2, r_o).astype(np.int16)
        gidx_t = np.tile(gidx.reshape(-1, 16).T, (8, 1))  # [128, total_tok//16]

        dis_l = dis[c * NPC:(c + 1) * NPC]     # position space
        dis_pad = np.zeros(NPAD, np.float32)
        dis_pad[:NPC] = dis_l
        dis_col = np.ascontiguousarray(dis_pad.reshape(NW, 128).T)  # [128, NW]
        dis2_col = dis_col * dis_col
        # Bstt[p, w*128+f] = dis[w*128+p] * b1[f]
        Bstt = (dis_col.T[:, :, None] * b1[None, None, :]).transpose(1, 0, 2)
        Bstt = np.ascontiguousarray(Bstt.reshape(128, NW * FHID))

        xT = np.zeros((FIN, NPAD), np.float32)
        xT[:, :NPC] = (x[c * NPC + perms[c]] * dis_l[:, None]).T

        in_maps.append({
            "gidx": np.ascontiguousarray(gidx_t),
            "xT": xT,
            "W1": W1, "W2": W2,
            "dis2c": dis2_col, "disc": dis_col,
            "Bstt": Bstt,
            "b2b": np.tile(b2, (128, 1)),
            "ident": np.eye(128, dtype=np.float32),
        })
    return in_maps, (KLOm, KHIm, bool(not b1.any()), bool(not b2.any())), perms


def _build(Kinfo):
    import concourse.bacc as bacc
    import concourse.mybir as mybir
    import concourse.tile as tile

    KLOm, KHIm, B1ZERO, B2ZERO = Kinfo
    K = KLOm + KHIm
    maxK = int(K.max())
    total_tok = int(K.sum()) * 128

    PHASES = os.environ.get("GCN_PHASES", "full")
    REPEAT = int(os.environ.get("GCN_REPEAT", "1"))
    SKIPGATHER = bool(os.environ.get("GCN_SKIPGATHER"))  # timing probe only
    SKIPREDUCE = bool(os.environ.get("GCN_SKIPREDUCE"))  # timing probe only
    SKIPAG = bool(os.environ.get("GCN_SKIPAG"))          # timing probe only
    MR = int(os.environ.get("GCN_MAXRANKS", str(MAXRANKS)))

    dt = mybir.dt
    ALU = mybir.AluOpType

    nc = bacc.Bacc("TRN2", target_bir_lowering=False, debug=False,
                   num_devices=NCORES)

    gidx_d = nc.dram_tensor("gidx", [128, total_tok // 16], dt.int16, kind="ExternalInput")
    xT_d = nc.dram_tensor("xT", [FIN, NPAD], dt.float32, kind="ExternalInput")
    W1_d = nc.dram_tensor("W1", [FIN, FHID], dt.float32, kind="ExternalInput")
    W2_d = nc.dram_tensor("W2", [FHID, FOUT], dt.float32, kind="ExternalInput")
    dis2_d = nc.dram_tensor("dis2c", [128, NW], dt.float32, kind="ExternalInput")
    dis_d = nc.dram_tensor("disc", [128, NW], dt.float32, kind="ExternalInput")
    Bstt_d = nc.dram_tensor("Bstt", [128, NW * FHID], dt.float32, kind="ExternalInput")
    b2b_d = nc.dram_tensor("b2b", [128, FOUT], dt.float32, kind="ExternalInput")
    ident_d = nc.dram_tensor("ident", [128, 128], dt.float32, kind="ExternalInput")
    out_d = nc.dram_tensor("out", [NPC, FOUT], dt.float32, kind="ExternalOutput")

    # double-buffered across repeats: repeat r+1's phase B / AllGather can
    # overlap repeat r's L2 gathers instead of serializing on a WAR hazard
    t1_locals = [nc.dram_tensor(f"t1_local{i}", [NPC2, FHID], dt.float32)
                 for i in range(2)]
    t1_fulls = [nc.dram_tensor(f"t1_full{i}", [NFULL, FHID], dt.float32,
                               addr_space="Shared") for i in range(2)]
    t2_locals = [nc.dram_tensor(f"t2_local{i}", [NPC2, FOUT], dt.float32)
                 for i in range(2)]
    t2_fulls = [nc.dram_tensor(f"t2_full{i}", [NFULL, FOUT], dt.float32,
                               addr_space="Shared") for i in range(2)]

    NWF = NW - 1  # 48 full windows; window 48 has 106 live rows

    with tile.TileContext(nc) as tc:
        with (
            tc.tile_pool(name="consts", bufs=1) as cp,
            tc.tile_pool(name="work", bufs=1) as wp,
            tc.tile_pool(name="psum", bufs=1, space="PSUM") as pp,
        ):
            ident_t = cp.tile([128, 128], dt.float32, tag="ident")
            nc.sync.dma_start(ident_t[:], ident_d[:, :])
            w1_t = cp.tile([128, 2, FHID], dt.float32, tag="w1")
            nc.sync.dma_start(w1_t[:, 0, :], W1_d[0:128, :])
            nc.sync.dma_start(w1_t[:, 1, :], W1_d[128:256, :])
            w2_t = cp.tile([FHID, FOUT], dt.float32, tag="w2")
            nc.sync.dma_start(w2_t[:], W2_d[:, :])
            dis2_t = cp.tile([128, NW], dt.float32, tag="dis2")
            nc.sync.dma_start(dis2_t[:], dis2_d[:, :])
            dis_t = cp.tile([128, NW], dt.float32, tag="dis")
            nc.sync.dma_start(dis_t[:], dis_d[:, :])
            if not B1ZERO:
                Bstt_t = cp.tile([128, NW * FHID], dt.float32, tag="Bstt")
                nc.sync.dma_start(Bstt_t[:], Bstt_d[:, :])
            b2b_t = cp.tile([128, FOUT], dt.float32, tag="b2b")
            nc.sync.dma_start(b2b_t[:], b2b_d[:, :])
            gidx_t = cp.tile([128, total_tok // 16], dt.int16, tag="gidx")
            nc.sync.dma_start(gidx_t[:], gidx_d[:, :])
            zrow = cp.tile([128, FHID], dt.float32, tag="zrow")
            nc.vector.memset(zrow[:], 0.0)

            # one shared gpsimd register per distinct gather count: avoids a
            # RegisterMove instruction (~55us here) per dma_gather
            counts = set()
            for w in range(NW):
                for nk in (int(KLOm[w]), int(KHIm[w])):
                    for k0 in range(0, nk, MR):
                        counts.add(min(MR, nk - k0) * 128)
            nidx_regs = {cnt: nc.gpsimd.to_reg(cnt) for cnt in sorted(counts)}

            dis2_bc = dis2_t[:].rearrange("p (w o) -> p w o", o=1) \
                               .broadcast_to([128, NW, FHID])
            dis_bc = dis_t[:].rearrange("p (w o) -> p w o", o=1) \
                             .broadcast_to([128, NW, FOUT])
            b2_bc = b2b_t[:].rearrange("(o p) f -> p o f", o=1) \
                            .broadcast_to([128, NW, FOUT])

            for _rep in range(REPEAT):
                t1_local, t1_full = t1_locals[_rep % 2], t1_fulls[_rep % 2]
                t2_local, t2_full = t2_locals[_rep % 2], t2_fulls[_rep % 2]
                # ---- phase B: t1_local = (dis*x) @ W1 ----
                with tc.tile_pool(name="phaseB", bufs=1) as pb:
                    xT_t = pb.tile([128, 2, NPAD], dt.float32, tag="xT")
                    nc.sync.dma_start(xT_t[:, 0, :], xT_d[0:128, :])
                    nc.sync.dma_start(xT_t[:, 1, :], xT_d[128:256, :])
                    evB = pb.tile([128, NWF, FHID], dt.float32, tag="evB")
                    evBt = pb.tile([128, FHID], dt.float32, tag="evBt")
                    psB = pp.tile([128, 8, FHID], dt.float32, tag="pB")
                    for w in range(NW):
                        sl = psB[:, w % 8, :]
                        for kc in range(2):
                            nc.tensor.matmul(
                                sl, xT_t[:, kc, w * 128:w * 128 + 128],
                                w1_t[:, kc, :], start=(kc == 0), stop=(kc == 1))
                        if w % 8 == 7:
                            nc.vector.tensor_copy(evB[:, w - 7:w + 1, :], psB[:])
                        if w == NW - 1:
                            nc.vector.tensor_copy(evBt[:], sl)
                    nc.sync.dma_start(
                        t1_local[0:NWF * 128, :]
                        .rearrange("(a p) f -> p a f", p=128), evB[:])
                    nc.sync.dma_start(t1_local[NWF * 128:NPC, :],
                                      evBt[0:NPC - NWF * 128, :])
                    nc.sync.dma_start(t1_local[NPC:NPC2, :], zrow[0:1, :])

                if SKIPAG:
                    nc.sync.dma_start(t1_full[0:NPC2, :], t1_local[:, :])
                else:
                    nc.gpsimd.collective_compute(
                        "AllGather", mybir.AluOpType.bypass,
                        replica_groups=[list(range(NCORES))],
                        ins=[t1_local[:, :]], outs=[t1_full[:, :]],
                    )

                if PHASES == "Bdump":
                    # debug: out <- first FOUT cols of t1_local
                    nc.sync.dma_start(out_d[:, :], t1_local[0:NPC, 0:FOUT])
                    continue

                if PHASES == "B":
                    ot = wp.tile([128, FOUT], dt.float32, tag="o")
                    nc.vector.memset(ot[:], 0.0)
                    for w in range(NW):
                        rows = min(128, NPC - w * 128)
                        nc.sync.dma_start(out_d[w * 128:w * 128 + rows, :],
                                          ot[0:rows, :])
                    continue

                def gather_window(tok, w, src_full, feat, woff_w, dk=0):
                    """Emit gathers for window w into tok at rank offset dk."""
                    klo, khi = int(KLOm[w]), int(KHIm[w])
                    base_lo = src_full[0:HALF, :]
                    base_hi = src_full[H2:NFULL, :]
                    segs = [(0, klo, base_lo), (klo, khi, base_hi)]
                    for seg0, nk, base in segs:
                        for k0 in range(0, nk, MR):
                            kn = min(MR, nk - k0)
                            c0 = (woff_w + (seg0 + k0) * 128) // 16
                            d0 = dk + seg0 + k0
                            if SKIPGATHER:
                                continue
                            nc.gpsimd.dma_gather(
                                tok[:, d0:d0 + kn, :], base,
                                gidx_t[:, c0:c0 + kn * 8],
                                num_idxs=kn * 128,
                                num_idxs_reg=nidx_regs[kn * 128],
                                elem_size=feat,
                                single_packet=(kn * 128 <= 1024))

                # ---- L1 pass 1: gather + reduce into red_all, one fused
                #      relu+scale over all windows ----
                with tc.tile_pool(name="L1", bufs=1) as l1:
                    tok1a = l1.tile([128, maxK, FHID], dt.float32, tag="tok1a")
                    tok1b = l1.tile([128, maxK, FHID], dt.float32, tag="tok1b")
                    toks = [tok1a, tok1b]
                    red_all = l1.tile([128, NW, FHID], dt.float32, tag="redA")
                    o1s_all = l1.tile([128, NW, FHID], dt.float32, tag="o1sa")
                    o1T = l1.tile([128, 4, FHID], dt.float32, tag="o1T")
                    ev1 = l1.tile([128, NWF, FOUT], dt.float32, tag="ev1")
                    ev1t = l1.tile([128, FOUT], dt.float32, tag="ev1t")
                    pT = pp.tile([128, 4, 512], dt.float32, tag="pT")  # slice per bank
                    p2 = pp.tile([128, 8, FOUT], dt.float32, tag="p2")
                    if SKIPGATHER:
                        nc.vector.memset(toks[0][:], 0.0)
                        nc.vector.memset(toks[1][:], 0.0)
                    if SKIPREDUCE:
                        nc.vector.memset(red_all[:], 0.0)
                    woff_w = 0
                    for w in range(NW):
                        tok = toks[w % 2]
                        gather_window(tok, w, t1_full, FHID, woff_w, 0)
                        woff_w += int(K[w]) * 128
                        if not SKIPREDUCE:
                            nc.vector.tensor_reduce(
                                red_all[:, w:w + 1, :],
                                tok[:, 0:int(K[w]), :]
                                .rearrange("p (b k) f -> p b f k", b=1),
                                mybir.AxisListType.X, ALU.add)
                    # self-loop contribution: red += own t1 rows
                    own1 = l1.tile([128, NW, FHID], dt.float32, tag="own1")
                    nc.vector.memset(own1[:], 0.0)
                    nc.sync.dma_start(
                        own1[:, 0:NWF, :],
                        t1_local[0:NWF * 128, :]
                        .rearrange("(a p) f -> p a f", p=128))
                    nc.sync.dma_start(own1[0:NPC - NWF * 128, NWF, :],
                                      t1_local[NWF * 128:NPC, :])
                    nc.vector.scalar_tensor_tensor(
                        red_all[:], red_all[:], 0.0, own1[:],
                        ALU.bypass, ALU.add)
                    # o1s = relu(dis^2*red + dis*b1), batched over all windows
                    if B1ZERO:
                        if os.environ.get("GCN_NOBCAST"):
                            for w in range(NW):
                                nc.vector.tensor_scalar(
                                    o1s_all[:, w, :], red_all[:, w, :],
                                    dis2_t[:, w:w + 1], 0.0, ALU.mult, ALU.max)
                        else:
                            nc.vector.scalar_tensor_tensor(
                                o1s_all[:], red_all[:], 0.0, dis2_bc,
                                ALU.max, ALU.mult)
                    else:
                        nc.vector.scalar_tensor_tensor(
                            o1s_all[:], red_all[:], 0.0, dis2_bc,
                            ALU.bypass, ALU.mult)
                        nc.vector.scalar_tensor_tensor(
                            o1s_all[:], o1s_all[:], 0.0,
                            Bstt_t[:].rearrange("p (w f) -> p w f", w=NW),
                            ALU.bypass, ALU.add)
                        nc.vector.tensor_scalar(
                            o1s_all[:], o1s_all[:], 0.0, None, ALU.max)
                    # ---- L1 pass 2: transpose + @W2, batched ----
                    for w in range(NW):
                        nc.tensor.transpose(pT[:, w % 4, 0:FHID],
                                            o1s_all[:, w, :], ident_t[:])
                        if w % 4 == 3:
                            nc.vector.tensor_copy(o1T[:], pT[:, :, 0:FHID])
                        if w == NW - 1:
                            nc.vector.tensor_copy(o1T[:, 0, :], pT[:, 0, 0:FHID])
                        if w % 4 == 3 or w == NW - 1:
                            for w2 in range(w - (3 if w % 4 == 3 else 0), w + 1):
                                nc.tensor.matmul(p2[:, w2 % 8, :],
                                                 o1T[:, w2 % 4, :], w2_t[:],
                                                 start=True, stop=True)
                        if w % 8 == 7:
                            nc.vector.tensor_copy(ev1[:, w - 7:w + 1, :], p2[:])
                        if w == NW - 1:
                            nc.vector.tensor_copy(ev1t[:], p2[:, 0, :])
                    nc.sync.dma_start(
                        t2_local[0:NWF * 128, :]
                        .rearrange("(a p) f -> p a f", p=128), ev1[:])
                    nc.sync.dma_start(t2_local[NWF * 128:NPC, :],
                                      ev1t[0:NPC - NWF * 128, :])
                    nc.sync.dma_start(t2_local[NPC:NPC2, :], zrow[0:1, 0:FOUT])

                if PHASES == "B1dump":
                    nc.sync.dma_start(out_d[:, :], t2_local[0:NPC, :])
                    continue

                if PHASES == "B1":
                    ot = wp.tile([128, FOUT], dt.float32, tag="o")
                    nc.vector.memset(ot[:], 0.0)
                    for w in range(NW):
                        rows = min(128, NPC - w * 128)
                        nc.sync.dma_start(out_d[w * 128:w * 128 + rows, :],
                                          ot[0:rows, :])
                    continue

                if SKIPAG:
                    nc.sync.dma_start(t2_full[0:NPC2, :], t2_local[:, :])
                else:
                    nc.gpsimd.collective_compute(
                        "AllGather", mybir.AluOpType.bypass,
                        replica_groups=[list(range(NCORES))],
                        ins=[t2_local[:, :]], outs=[t2_full[:, :]],
                    )

                # ---- L2 windows ----
                with tc.tile_pool(name="L2", bufs=1) as l2:
                    tok2a = l2.tile([128, maxK, FOUT], dt.float32, tag="tok2a")
                    tok2b = l2.tile([128, maxK, FOUT], dt.float32, tag="tok2b")
                    tok2s = [tok2a, tok2b]
                    red2_all = l2.tile([128, NW, FOUT], dt.float32, tag="red2A")
                    ev2_all = l2.tile([128, NW, FOUT], dt.float32, tag="ev2A")
                    if SKIPGATHER:
                        nc.vector.memset(tok2s[0][:], 0.0)
                        nc.vector.memset(tok2s[1][:], 0.0)
                    if SKIPREDUCE:
                        nc.vector.memset(red2_all[:], 0.0)
                    woff_w = 0
                    for w in range(NW):
                        tok2 = tok2s[w % 2]
                        gather_window(tok2, w, t2_full, FOUT, woff_w, 0)
                        woff_w += int(K[w]) * 128
                        if not SKIPREDUCE:
                            nc.vector.tensor_reduce(
                                red2_all[:, w:w + 1, :],
                                tok2[:, 0:int(K[w]), :]
                                .rearrange("p (b k) f -> p b f k", b=1),
                                mybir.AxisListType.X, ALU.add)
                    # self-loop contribution: red2 += own t2 rows
                    own2 = l2.tile([128, NW, FOUT], dt.float32, tag="own2")
                    nc.vector.memset(own2[:], 0.0)
                    nc.sync.dma_start(
                        own2[:, 0:NWF, :],
                        t2_local[0:NWF * 128, :]
                        .rearrange("(a p) f -> p a f", p=128))
                    nc.sync.dma_start(own2[0:NPC - NWF * 128, NWF, :],
                                      t2_local[NWF * 128:NPC, :])
                    nc.vector.scalar_tensor_tensor(
                        red2_all[:], red2_all[:], 0.0, own2[:],
                        ALU.bypass, ALU.add)
                    # out = dis*red2 (+ b2), batched over all windows
                    if os.environ.get("GCN_NOBCAST"):
                        for w in range(NW):
                            nc.vector.scalar_tensor_tensor(
                                ev2_all[:, w, :], red2_all[:, w, :],
                                dis_t[:, w:w + 1], b2b_t[:],
                                ALU.mult, ALU.add)
                    else:
                        nc.vector.scalar_tensor_tensor(
                            ev2_all[:], red2_all[:], 0.0, dis_bc,
                            ALU.bypass, ALU.mult)
                        if not B2ZERO:
                            nc.vector.scalar_tensor_tensor(
                                ev2_all[:], ev2_all[:], 0.0, b2_bc,
                                ALU.bypass, ALU.add)
                    nc.sync.dma_start(
                        out_d[0:NWF * 128, :]
                        .rearrange("(a p) f -> p a f", p=128),
                        ev2_all[:, 0:NWF, :])
                    nc.sync.dma_start(out_d[NWF * 128:NPC, :],
                                      ev2_all[0:NPC - NWF * 128, NWF, :])

    nc.compile()
    return nc


def kernel(x, edge_index, W1, b1, W2, b2):
    global LAST_RESULTS
    from concourse.bass_utils import run_bass_kernel_spmd

    in_maps, Kinfo, perms = _host_prep(x, edge_index, W1, b1, W2, b2)
    key = (Kinfo[0].tobytes(), Kinfo[1].tobytes(), Kinfo[2], Kinfo[3])
    if key not in _CACHE:
        _CACHE[key] = _build(Kinfo)
    nc = _CACHE[key]

    res = run_bass_kernel_spmd(nc, in_maps, list(range(NCORES)))
    LAST_RESULTS = res
    out = np.empty((N, FOUT), np.float32)
    for c in range(NCORES):
        out[c * NPC + perms[c]] = res.results[c]["out"]
    return out
